# revision 10
# baseline (speedup 1.0000x reference)
"""Bilateral filter (7x7, dilation 1) Trainium2 Bass kernel — v2.

Problem: input [2, 18, 1024, 1024] f32.
  filterable = input[:, :8]; params = input[:, 8:]
  r_c = -(p_c^2), sx = -(p8^2), sy = -(p9^2)
  logw = sum_c r_c (fn_c - f_c)^2 + sx dx^2 + sy dy^2  (OOB taps masked)
  out[c] = sum_taps w * fn_c / sum_taps w,  c < 3

Sharding: data-parallel over (batch, H): 8 cores, each 256 rows of one batch
image (+3 halo rows each side, sentinel-padded host-side, sentinel=100).

v2 design (per core, 2 row-blocks x 2 W-chunks of [128 rows, 512 cols]):
  - fp16 on-chip compute (DVE 2x_1P mode for all tensor_tensor ops),
    channel-planar ("c-major") free-axis layout [128, 8ch * 518cols] so every
    hot AP is unit-stride.
  - GPSIMD cast-DMA (f32->f16) loads a 134-row staging set; 7 row-shifted
    tile copies made with partition-shifted SBUF->SBUF DMAs on the SP queue.
  - Per tap: DVE sub; ACT Square (in-place); mult by p^2 split DVE/GPSIMD;
    pairwise-tree channel reduce on DVE (contiguous halves, 2x mode);
    + spatial term; clamp; ACT exp(scale=-1); w*fn3 (DVE/GPSIMD alternating);
    PE identity-matmul accumulates [w*fn3 | w] into PSUM across all 49 taps
    (fp32 accumulation for free on the otherwise idle tensor engine).
  - Weight math: w = exp(-(sum_c p_c^2 d_c^2 + a*sx^2 + b*sy^2)), all terms
    computed as positives.  Sentinel pixels drive the quadratic form to
    +huge (or +inf) -> exp -> +0, reproducing the reference's OOB mask.
"""

import sys

if "/opt/trn_rl_repo" not in sys.path:
    sys.path.insert(0, "/opt/trn_rl_repo")

import numpy as np

import concourse.bass as bass
import concourse.mybir as mybir
from concourse.bacc import Bacc
from concourse.tile import TileContext
from concourse.masks import make_identity

FP32 = mybir.dt.float32
FP16 = mybir.dt.float16
AF = mybir.ActivationFunctionType

B, C_ALL, H, W = 2, 18, 1024, 1024
CF = 8                      # filterable channels
CO = 3                      # output channels
KS, RAD = 7, 3
HC = H * B // 8             # 256 output rows per core
HIN = HC + 2 * RAD          # 262 input rows per core (halo padded host-side)
WC = 512                    # W chunk
NW = W // WC                # 2
NHB = HC // 128             # 2
WT = WC + 2 * RAD           # 518 (with column halo)
SENT = 8.0                  # sentinel: max quadratic form ~41K < fp16 max,
                            # so no inf on-chip, yet exp(-s) underflows to +0
D2IDX = [3, 2, 1, 0, 1, 2, 3]              # index into D2VALS: (k-3)^2
D2VALS = [0.0, 1.0, 4.0, 9.0]

# engine-split tuning knobs
MD = 6          # channels of the p^2*d^2 multiply done on DVE (rest GPSIMD)
GP_MUL3 = 1     # every GP_MUL3-th tap's w*fn3 runs on GPSIMD (0 = never)
CTR = KS // 2 * KS + KS // 2   # center tap index (w == 1 fast path)

_CACHED = {}


def _cm(ap, w=WT, c=CF):
    """View flat [128, c*w] as [128, c, w] (channel-major blocks)."""
    return ap.rearrange("p (c x) -> p c x", c=c)


def build_nc():
    nc = Bacc()
    x = nc.dram_tensor("x", [HIN, C_ALL, W], FP32, kind="ExternalInput")
    y = nc.dram_tensor("y", [CO, HC, W], FP32, kind="ExternalOutput")

    with TileContext(nc) as tc:
        with (
            tc.tile_pool(name="ipool", bufs=1) as ipool,
            tc.tile_pool(name="fpool", bufs=1) as fpool,
            tc.tile_pool(name="cpool", bufs=1) as cpool,
            tc.tile_pool(name="dpool", bufs=1) as dpool,
            tc.tile_pool(name="spool", bufs=1) as spool,
            tc.tile_pool(name="ppool", bufs=1, space="PSUM") as ppool,
        ):
            ident = ipool.tile([128, 128], FP16, tag="ident", name="ident")
            make_identity(nc, ident[:])
            for hb in range(NHB):
                for wck in range(NW):
                    _macro(nc, tc, x, y, ident, fpool, cpool, dpool, spool,
                           ppool, hb, wck)
    nc.compile()
    return nc


def _macro(nc, tc, x, y, ident, fpool, cpool, dpool, spool, ppool, hb, wck):
    w0 = wck * WC
    r0 = hb * 128
    # staging-tile col t <-> image col w0 - 3 + t
    lo = RAD if wck == 0 else 0
    hi = WT - RAD if wck == NW - 1 else WT

    # ---- staging: cast-DMA f32 -> f16, rows r0 .. r0+133 of the slab ----
    # Ct: slab rows r0..r0+2 | Fm: r0+3..r0+130 (= center tile F[3]) |
    # Cb: r0+131..r0+133
    Ct = fpool.tile([RAD, CF * WT], FP16, tag="Ct", name=f"Ct_{hb}_{wck}")
    Fm = fpool.tile([128, CF * WT], FP16, tag="Fm", name=f"Fm_{hb}_{wck}")
    Cb = fpool.tile([RAD, CF * WT], FP16, tag="Cb", name=f"Cb_{hb}_{wck}")

    for tile, rb, nr in ((Ct, r0, RAD), (Fm, r0 + RAD, 128),
                         (Cb, r0 + RAD + 128, RAD)):
        v = tile[:].rearrange("p (c x) -> p c x", c=CF)
        if lo > 0:
            nc.gpsimd.memset(v[:, :, 0:lo], SENT)
        if hi < WT:
            nc.gpsimd.memset(v[:, :, hi:WT], SENT)
        nc.gpsimd.dma_start(
            out=v[:, :, lo:hi],
            in_=x[rb : rb + nr, 0:CF, w0 - RAD + lo : w0 - RAD + hi],
        )

    # ---- 7 row-shifted tiles: F[oy][p] = staging row oy+p ----
    F = [None] * KS
    F[RAD] = Fm
    for oy in range(KS):
        if oy == RAD:
            continue
        Ft = fpool.tile([128, CF * WT], FP16, tag=f"F{oy}",
                        name=f"F{oy}_{hb}_{wck}")
        if oy < RAD:
            k = RAD - oy  # rows from Ct
            nc.sync.dma_start(out=Ft[0:k, :], in_=Ct[oy:RAD, :])
            nc.sync.dma_start(out=Ft[k:128, :], in_=Fm[0 : 128 - k, :])
        else:
            k = oy - RAD  # rows from Cb
            nc.sync.dma_start(out=Ft[0 : 128 - k, :], in_=Fm[k:128, :])
            nc.sync.dma_start(out=Ft[128 - k : 128, :], in_=Cb[0:k, :])
        F[oy] = Ft
    Fc = _cm(Fm[:])[:, :, RAD : RAD + WC]

    # ---- params: P2[c] = p_c^2 (f16, c-major), sx2/sy2 ----
    P2 = cpool.tile([128, CF * WC], FP16, tag="P2", name=f"P2_{hb}_{wck}")
    sxy2 = cpool.tile([128, 2 * WC], FP16, tag="sxy2", name=f"sxy2_{hb}_{wck}")
    for k in range(CF + 2):
        pst = fpool.tile([128, WC], FP32, tag="pst", bufs=2,
                         name=f"pst_{hb}_{wck}_{k}")
        nc.sync.dma_start(
            out=pst[:],
            in_=x[r0 + RAD : r0 + RAD + 128, CF + k, w0 : w0 + WC])
        dst = (P2[:, k * WC : (k + 1) * WC] if k < CF
               else sxy2[:, (k - CF) * WC : (k - CF + 1) * WC])
        nc.scalar.activation(dst, pst[:], AF.Square)
    sx2 = sxy2[:, 0:WC]
    sy2 = sxy2[:, WC : 2 * WC]

    # ---- spatial log-weights: asp(a, b) = a*sx2 + b*sy2 (positive) ----
    Aa = cpool.tile([128, 3 * WC], FP16, tag="Aa", name=f"Aa_{hb}_{wck}")
    Ab = cpool.tile([128, 3 * WC], FP16, tag="Ab", name=f"Ab_{hb}_{wck}")
    for ai in (1, 2, 3):
        nc.vector.tensor_scalar_mul(
            Aa[:, (ai - 1) * WC : ai * WC], sx2, float(D2VALS[ai]))
        nc.vector.tensor_scalar_mul(
            Ab[:, (ai - 1) * WC : ai * WC], sy2, float(D2VALS[ai]))
    Asum = cpool.tile([128, 9 * WC], FP16, tag="Asum", name=f"As_{hb}_{wck}")
    for ai in (1, 2, 3):
        for bi in (1, 2, 3):
            k = (ai - 1) * 3 + (bi - 1)
            nc.vector.tensor_add(
                Asum[:, k * WC : (k + 1) * WC],
                Aa[:, (ai - 1) * WC : ai * WC],
                Ab[:, (bi - 1) * WC : bi * WC])

    def asp_ap(i, j):
        ai, bi = D2IDX[j], D2IDX[i]   # x-dist from col shift j, y from row i
        if ai == 0 and bi == 0:
            return None
        if bi == 0:
            return Aa[:, (ai - 1) * WC : ai * WC]
        if ai == 0:
            return Ab[:, (bi - 1) * WC : bi * WC]
        k = (ai - 1) * 3 + (bi - 1)
        return Asum[:, k * WC : (k + 1) * WC]

    # ---- PSUM accumulator: [w*fn0 | w*fn1 | w*fn2 | w] ----
    ps = ppool.tile([128, 4 * WC], FP32, tag="ps", bufs=2,
                    name=f"ps_{hb}_{wck}")

    taps = [(i, j) for i in range(KS) for j in range(KS)]
    n = len(taps)
    Dt, Tt = {}, {}

    def stage_a(t):     # sub + square (in-place)
        if t == CTR:
            return
        i, j = taps[t]
        d = dpool.tile([128, CF * WC], FP16, tag="d", bufs=6,
                       name=f"d_{hb}_{wck}_{t}")
        nc.vector.tensor_sub(_cm(d[:], WC), _cm(F[i][:])[:, :, j : j + WC], Fc)
        # square: DVE for its own channels (keeps the chain on one engine),
        # ACT only for the GPSIMD-bound channels (starts GPSIMD's chain early)
        if MD > 0:
            nc.vector.tensor_mul(d[:, 0 : MD * WC], d[:, 0 : MD * WC],
                                 d[:, 0 : MD * WC])
        if MD < CF:
            nc.scalar.activation(d[:, MD * WC :], d[:, MD * WC :], AF.Square)
        Dt[t] = d

    def stage_m(t):     # p^2 multiply, split DVE / GPSIMD
        if t == CTR:
            return
        dv = Dt[t][:]
        if MD > 0:
            nc.vector.tensor_mul(dv[:, 0 : MD * WC], P2[:, 0 : MD * WC],
                                 dv[:, 0 : MD * WC])
        if MD < CF:
            nc.gpsimd.tensor_mul(dv[:, MD * WC :], P2[:, MD * WC :],
                                 dv[:, MD * WC :])

    def stage_r(t):     # tree-reduce, +asp, exp
        T = spool.tile([128, 4 * WC], FP16, tag="T", bufs=6,
                       name=f"T_{hb}_{wck}_{t}")
        Tt[t] = T
        if t == CTR:
            nc.gpsimd.memset(T[:, 3 * WC : 4 * WC], 1.0)
            return
        i, j = taps[t]
        dv = Dt.pop(t)[:]
        nc.vector.tensor_add(dv[:, 0 : 4 * WC], dv[:, 0 : 4 * WC],
                             dv[:, 4 * WC : 8 * WC])
        nc.vector.tensor_add(dv[:, 0 : 2 * WC], dv[:, 0 : 2 * WC],
                             dv[:, 2 * WC : 4 * WC])
        nc.vector.tensor_add(dv[:, 0:WC], dv[:, 0:WC], dv[:, WC : 2 * WC])
        ap = asp_ap(i, j)
        if ap is not None:
            nc.vector.tensor_add(dv[:, 0:WC], dv[:, 0:WC], ap)
        nc.scalar.activation(T[:, 3 * WC : 4 * WC], dv[:, 0:WC], AF.Exp,
                             scale=-1.0)

    def stage_c(t):     # w*fn3, then PE accumulates [w*fn3 | w] into PSUM
        i, j = taps[t]
        T = Tt.pop(t)
        fn3 = _cm(F[i][:])[:, 0:CO, j : j + WC]
        if t == CTR:
            nc.vector.tensor_copy(_cm(T[:, 0 : CO * WC], WC, CO), fn3)
        else:
            w_b = T[:, 3 * WC : 4 * WC].unsqueeze(1).broadcast_to(
                [128, CO, WC])
            eng = nc.gpsimd if (GP_MUL3 and t % GP_MUL3 == 0) else nc.vector
            eng.tensor_mul(_cm(T[:, 0 : CO * WC], WC, CO), w_b, fn3)
        for k in range(4):
            nc.tensor.matmul(
                ps[:, k * WC : (k + 1) * WC], ident[:],
                T[:, k * WC : (k + 1) * WC],
                start=(t == 0), stop=(t == n - 1))

    # issue order inside an iteration matters: exp (in stage_r) must precede
    # the next square (stage_a) in the ACT queue, else everything downstream
    # of exp serializes behind the 4.4us square.
    for t in range(n + 3):
        if 0 <= t - 2 < n:
            stage_r(t - 2)
        if t < n:
            stage_a(t)
        if 0 <= t - 1 < n:
            stage_m(t - 1)
        if 0 <= t - 3 < n:
            stage_c(t - 3)

    # ---- out = acc / wsum ----
    rec = spool.tile([128, WC], FP32, tag="rec", name=f"rec_{hb}_{wck}")
    nc.vector.reciprocal(rec[:], ps[:, 3 * WC : 4 * WC])
    out3 = spool.tile([128, CO * WC], FP32, tag="out3", name=f"o3_{hb}_{wck}")
    rec_b = rec[:].unsqueeze(1).broadcast_to([128, CO, WC])
    nc.vector.tensor_mul(_cm(out3[:], WC, CO), rec_b,
                         _cm(ps[:, 0 : CO * WC], WC, CO))
    for c in range(CO):
        nc.sync.dma_start(out=y[c, r0 : r0 + 128, w0 : w0 + WC],
                          in_=out3[:, c * WC : (c + 1) * WC])


def shard_inputs(input):
    """input [2,18,1024,1024] -> 8 per-core slabs [262, 18, 1024]."""
    input = np.asarray(input, dtype=np.float32)
    per_b = 4
    rows = H // per_b
    in_maps = []
    for core in range(8):
        b, q = divmod(core, per_b)
        r0 = q * rows
        slab = np.full((HIN, C_ALL, W), SENT, dtype=np.float32)
        s_lo = max(r0 - RAD, 0)
        s_hi = min(r0 + rows + RAD, H)
        slab[s_lo - (r0 - RAD) : s_hi - (r0 - RAD), :, :] = (
            input[b, :, s_lo:s_hi, :].transpose(1, 0, 2))
        in_maps.append({"x": np.ascontiguousarray(slab)})
    return in_maps


def assemble(results):
    out = np.empty((B, CO, H, W), dtype=np.float32)
    rows = H // 4
    for core in range(8):
        b, q = divmod(core, 4)
        out[b, :, q * rows : (q + 1) * rows, :] = results[core]["y"]
    return out


def kernel(input):
    from concourse.bass_utils import run_bass_kernel_spmd

    if "nc" not in _CACHED:
        _CACHED["nc"] = build_nc()
    in_maps = shard_inputs(input)
    res = run_bass_kernel_spmd(_CACHED["nc"], in_maps, list(range(8)))
    return assemble(res.results)


# revision 12
# speedup vs baseline: 1.6816x; 1.6816x over previous
"""Bilateral filter (7x7, dilation 1) Trainium2 Bass kernel — v2.

Problem: input [2, 18, 1024, 1024] f32.
  filterable = input[:, :8]; params = input[:, 8:]
  r_c = -(p_c^2), sx = -(p8^2), sy = -(p9^2)
  logw = sum_c r_c (fn_c - f_c)^2 + sx dx^2 + sy dy^2  (OOB taps masked)
  out[c] = sum_taps w * fn_c / sum_taps w,  c < 3

Sharding: data-parallel over (batch, H): 8 cores, each 256 rows of one batch
image (+3 halo rows each side, sentinel-padded host-side, sentinel=100).

v2 design (per core, 2 row-blocks x 2 W-chunks of [128 rows, 512 cols]):
  - fp16 on-chip compute (DVE 2x_1P mode for all tensor_tensor ops),
    channel-planar ("c-major") free-axis layout [128, 8ch * 518cols] so every
    hot AP is unit-stride.
  - GPSIMD cast-DMA (f32->f16) loads a 134-row staging set; 7 row-shifted
    tile copies made with partition-shifted SBUF->SBUF DMAs on the SP queue.
  - Per tap: DVE sub; ACT Square (in-place); mult by p^2 split DVE/GPSIMD;
    pairwise-tree channel reduce on DVE (contiguous halves, 2x mode);
    + spatial term; clamp; ACT exp(scale=-1); w*fn3 (DVE/GPSIMD alternating);
    PE identity-matmul accumulates [w*fn3 | w] into PSUM across all 49 taps
    (fp32 accumulation for free on the otherwise idle tensor engine).
  - Weight math: w = exp(-(sum_c p_c^2 d_c^2 + a*sx^2 + b*sy^2)), all terms
    computed as positives.  Sentinel pixels drive the quadratic form to
    +huge (or +inf) -> exp -> +0, reproducing the reference's OOB mask.
"""

import sys

if "/opt/trn_rl_repo" not in sys.path:
    sys.path.insert(0, "/opt/trn_rl_repo")

import numpy as np

import concourse.bass as bass
import concourse.mybir as mybir
from concourse.bacc import Bacc
from concourse.tile import TileContext
from concourse.masks import make_identity

FP32 = mybir.dt.float32
FP16 = mybir.dt.float16
AF = mybir.ActivationFunctionType

B, C_ALL, H, W = 2, 18, 1024, 1024
CF = 8                      # filterable channels
CO = 3                      # output channels
KS, RAD = 7, 3
HC = H * B // 8             # 256 output rows per core
HIN = HC + 2 * RAD          # 262 input rows per core (halo padded host-side)
WC = 512                    # W chunk
NW = W // WC                # 2
NHB = HC // 128             # 2
WT = WC + 2 * RAD           # 518 (with column halo)
SENT = 8.0                  # sentinel: max quadratic form ~41K < fp16 max,
                            # so no inf on-chip, yet exp(-s) underflows to +0
D2IDX = [3, 2, 1, 0, 1, 2, 3]              # index into D2VALS: (k-3)^2
D2VALS = [0.0, 1.0, 4.0, 9.0]

# engine-split tuning knobs.  GPSIMD tensor ops share the DVE SBUF port and
# measurably throttle DVE (~1.8x slowdown while active), so the hot loop is
# DVE+ACT only.
MD = 8          # channels of the p^2*d^2 multiply done on DVE (rest GPSIMD)
GP_MUL3 = 0     # every GP_MUL3-th tap's w*fn3 runs on GPSIMD (0 = never)
CTR = KS // 2 * KS + KS // 2   # center tap index (w == 1 fast path)

_CACHED = {}


def _cm(ap, w=WT, c=CF):
    """View flat [128, c*w] as [128, c, w] (channel-major blocks)."""
    return ap.rearrange("p (c x) -> p c x", c=c)


def build_nc():
    nc = Bacc()
    x = nc.dram_tensor("x", [HIN, C_ALL, W], FP32, kind="ExternalInput")
    y = nc.dram_tensor("y", [CO, HC, W], FP32, kind="ExternalOutput")

    with TileContext(nc) as tc:
        with (
            tc.tile_pool(name="ipool", bufs=1) as ipool,
            tc.tile_pool(name="fpool", bufs=1) as fpool,
            tc.tile_pool(name="cpool", bufs=1) as cpool,
            tc.tile_pool(name="dpool", bufs=1) as dpool,
            tc.tile_pool(name="spool", bufs=1) as spool,
            tc.tile_pool(name="ppool", bufs=1, space="PSUM") as ppool,
        ):
            ident = ipool.tile([128, 128], FP16, tag="ident", name="ident")
            make_identity(nc, ident[:])
            for hb in range(NHB):
                for wck in range(NW):
                    _macro(nc, tc, x, y, ident, fpool, cpool, dpool, spool,
                           ppool, hb, wck)
    nc.compile()
    return nc


def _macro(nc, tc, x, y, ident, fpool, cpool, dpool, spool, ppool, hb, wck):
    w0 = wck * WC
    r0 = hb * 128
    # staging-tile col t <-> image col w0 - 3 + t
    lo = RAD if wck == 0 else 0
    hi = WT - RAD if wck == NW - 1 else WT

    # ---- staging: cast-DMA f32 -> f16, rows r0 .. r0+133 of the slab ----
    # Ct: slab rows r0..r0+2 | Fm: r0+3..r0+130 (= center tile F[3]) |
    # Cb: r0+131..r0+133
    Ct = fpool.tile([RAD, CF * WT], FP16, tag="Ct", name=f"Ct_{hb}_{wck}")
    Fm = fpool.tile([128, CF * WT], FP16, tag="Fm", name=f"Fm_{hb}_{wck}")
    Cb = fpool.tile([RAD, CF * WT], FP16, tag="Cb", name=f"Cb_{hb}_{wck}")

    for tile, rb, nr in ((Ct, r0, RAD), (Fm, r0 + RAD, 128),
                         (Cb, r0 + RAD + 128, RAD)):
        v = tile[:].rearrange("p (c x) -> p c x", c=CF)
        if lo > 0:
            nc.gpsimd.memset(v[:, :, 0:lo], SENT)
        if hi < WT:
            nc.gpsimd.memset(v[:, :, hi:WT], SENT)
        nc.gpsimd.dma_start(
            out=v[:, :, lo:hi],
            in_=x[rb : rb + nr, 0:CF, w0 - RAD + lo : w0 - RAD + hi],
        )

    # ---- 7 row-shifted tiles: F[oy][p] = staging row oy+p ----
    F = [None] * KS
    F[RAD] = Fm
    for oy in range(KS):
        if oy == RAD:
            continue
        Ft = fpool.tile([128, CF * WT], FP16, tag=f"F{oy}",
                        name=f"F{oy}_{hb}_{wck}")
        if oy < RAD:
            k = RAD - oy  # rows from Ct
            nc.sync.dma_start(out=Ft[0:k, :], in_=Ct[oy:RAD, :])
            nc.sync.dma_start(out=Ft[k:128, :], in_=Fm[0 : 128 - k, :])
        else:
            k = oy - RAD  # rows from Cb
            nc.sync.dma_start(out=Ft[0 : 128 - k, :], in_=Fm[k:128, :])
            nc.sync.dma_start(out=Ft[128 - k : 128, :], in_=Cb[0:k, :])
        F[oy] = Ft
    Fc = _cm(Fm[:])[:, :, RAD : RAD + WC]

    # ---- params: P2[c] = p_c^2 (f16, c-major), sx2/sy2 ----
    P2 = cpool.tile([128, CF * WC], FP16, tag="P2", name=f"P2_{hb}_{wck}")
    sxy2 = cpool.tile([128, 2 * WC], FP16, tag="sxy2", name=f"sxy2_{hb}_{wck}")
    for k in range(CF + 2):
        pst = fpool.tile([128, WC], FP32, tag="pst", bufs=2,
                         name=f"pst_{hb}_{wck}_{k}")
        nc.sync.dma_start(
            out=pst[:],
            in_=x[r0 + RAD : r0 + RAD + 128, CF + k, w0 : w0 + WC])
        dst = (P2[:, k * WC : (k + 1) * WC] if k < CF
               else sxy2[:, (k - CF) * WC : (k - CF + 1) * WC])
        nc.scalar.activation(dst, pst[:], AF.Square)
    sx2 = sxy2[:, 0:WC]
    sy2 = sxy2[:, WC : 2 * WC]

    # ---- spatial log-weights: asp(a, b) = a*sx2 + b*sy2 (positive) ----
    Aa = cpool.tile([128, 3 * WC], FP16, tag="Aa", name=f"Aa_{hb}_{wck}")
    Ab = cpool.tile([128, 3 * WC], FP16, tag="Ab", name=f"Ab_{hb}_{wck}")
    for ai in (1, 2, 3):
        nc.vector.tensor_scalar_mul(
            Aa[:, (ai - 1) * WC : ai * WC], sx2, float(D2VALS[ai]))
        nc.vector.tensor_scalar_mul(
            Ab[:, (ai - 1) * WC : ai * WC], sy2, float(D2VALS[ai]))
    Asum = cpool.tile([128, 9 * WC], FP16, tag="Asum", name=f"As_{hb}_{wck}")
    for ai in (1, 2, 3):
        for bi in (1, 2, 3):
            k = (ai - 1) * 3 + (bi - 1)
            nc.vector.tensor_add(
                Asum[:, k * WC : (k + 1) * WC],
                Aa[:, (ai - 1) * WC : ai * WC],
                Ab[:, (bi - 1) * WC : bi * WC])

    def asp_ap(i, j):
        ai, bi = D2IDX[j], D2IDX[i]   # x-dist from col shift j, y from row i
        if ai == 0 and bi == 0:
            return None
        if bi == 0:
            return Aa[:, (ai - 1) * WC : ai * WC]
        if ai == 0:
            return Ab[:, (bi - 1) * WC : bi * WC]
        k = (ai - 1) * 3 + (bi - 1)
        return Asum[:, k * WC : (k + 1) * WC]

    # ---- PSUM accumulator: [w*fn0 | w*fn1 | w*fn2 | w] ----
    ps = ppool.tile([128, 4 * WC], FP32, tag="ps", bufs=2,
                    name=f"ps_{hb}_{wck}")

    taps = [(i, j) for i in range(KS) for j in range(KS)]
    n = len(taps)
    Dt, Tt = {}, {}

    def stage_a(t):     # sub + square (in-place)
        if t == CTR:
            return
        i, j = taps[t]
        d = dpool.tile([128, CF * WC], FP16, tag="d", bufs=6,
                       name=f"d_{hb}_{wck}_{t}")
        nc.vector.tensor_sub(_cm(d[:], WC), _cm(F[i][:])[:, :, j : j + WC], Fc)
        nc.scalar.activation(d[:], d[:], AF.Square)
        Dt[t] = d

    def stage_m(t):     # p^2 multiply, split DVE / GPSIMD
        if t == CTR:
            return
        dv = Dt[t][:]
        if MD > 0:
            nc.vector.tensor_mul(dv[:, 0 : MD * WC], P2[:, 0 : MD * WC],
                                 dv[:, 0 : MD * WC])
        if MD < CF:
            nc.gpsimd.tensor_mul(dv[:, MD * WC :], P2[:, MD * WC :],
                                 dv[:, MD * WC :])

    def stage_r(t):     # tree-reduce, +asp, exp
        T = spool.tile([128, 4 * WC], FP16, tag="T", bufs=6,
                       name=f"T_{hb}_{wck}_{t}")
        Tt[t] = T
        if t == CTR:
            nc.gpsimd.memset(T[:, 3 * WC : 4 * WC], 1.0)
            return
        i, j = taps[t]
        dv = Dt.pop(t)[:]
        nc.vector.tensor_add(dv[:, 0 : 4 * WC], dv[:, 0 : 4 * WC],
                             dv[:, 4 * WC : 8 * WC])
        nc.vector.tensor_add(dv[:, 0 : 2 * WC], dv[:, 0 : 2 * WC],
                             dv[:, 2 * WC : 4 * WC])
        nc.vector.tensor_add(dv[:, 0:WC], dv[:, 0:WC], dv[:, WC : 2 * WC])
        ap = asp_ap(i, j)
        if ap is not None:
            nc.vector.tensor_add(dv[:, 0:WC], dv[:, 0:WC], ap)
        nc.scalar.activation(T[:, 3 * WC : 4 * WC], dv[:, 0:WC], AF.Exp,
                             scale=-1.0)

    def stage_c(t):     # w*fn3, then PE accumulates [w*fn3 | w] into PSUM
        i, j = taps[t]
        T = Tt.pop(t)
        fn3 = _cm(F[i][:])[:, 0:CO, j : j + WC]
        if t == CTR:
            nc.vector.tensor_copy(_cm(T[:, 0 : CO * WC], WC, CO), fn3)
        else:
            w_b = T[:, 3 * WC : 4 * WC].unsqueeze(1).broadcast_to(
                [128, CO, WC])
            eng = nc.gpsimd if (GP_MUL3 and t % GP_MUL3 == 0) else nc.vector
            eng.tensor_mul(_cm(T[:, 0 : CO * WC], WC, CO), w_b, fn3)
        for k in range(4):
            nc.tensor.matmul(
                ps[:, k * WC : (k + 1) * WC], ident[:],
                T[:, k * WC : (k + 1) * WC],
                start=(t == 0), stop=(t == n - 1))

    # issue order inside an iteration matters: exp (in stage_r) must precede
    # the next square (stage_a) in the ACT queue, else everything downstream
    # of exp serializes behind the 4.4us square.
    for t in range(n + 3):
        if 0 <= t - 2 < n:
            stage_r(t - 2)
        if t < n:
            stage_a(t)
        if 0 <= t - 1 < n:
            stage_m(t - 1)
        if 0 <= t - 3 < n:
            stage_c(t - 3)

    # ---- out = acc / wsum ----
    rec = spool.tile([128, WC], FP32, tag="rec", name=f"rec_{hb}_{wck}")
    nc.vector.reciprocal(rec[:], ps[:, 3 * WC : 4 * WC])
    out3 = spool.tile([128, CO * WC], FP32, tag="out3", name=f"o3_{hb}_{wck}")
    rec_b = rec[:].unsqueeze(1).broadcast_to([128, CO, WC])
    nc.vector.tensor_mul(_cm(out3[:], WC, CO), rec_b,
                         _cm(ps[:, 0 : CO * WC], WC, CO))
    for c in range(CO):
        nc.sync.dma_start(out=y[c, r0 : r0 + 128, w0 : w0 + WC],
                          in_=out3[:, c * WC : (c + 1) * WC])


def shard_inputs(input):
    """input [2,18,1024,1024] -> 8 per-core slabs [262, 18, 1024]."""
    input = np.asarray(input, dtype=np.float32)
    per_b = 4
    rows = H // per_b
    in_maps = []
    for core in range(8):
        b, q = divmod(core, per_b)
        r0 = q * rows
        slab = np.full((HIN, C_ALL, W), SENT, dtype=np.float32)
        s_lo = max(r0 - RAD, 0)
        s_hi = min(r0 + rows + RAD, H)
        slab[s_lo - (r0 - RAD) : s_hi - (r0 - RAD), :, :] = (
            input[b, :, s_lo:s_hi, :].transpose(1, 0, 2))
        in_maps.append({"x": np.ascontiguousarray(slab)})
    return in_maps


def assemble(results):
    out = np.empty((B, CO, H, W), dtype=np.float32)
    rows = H // 4
    for core in range(8):
        b, q = divmod(core, 4)
        out[b, :, q * rows : (q + 1) * rows, :] = results[core]["y"]
    return out


def kernel(input):
    from concourse.bass_utils import run_bass_kernel_spmd

    if "nc" not in _CACHED:
        _CACHED["nc"] = build_nc()
    in_maps = shard_inputs(input)
    res = run_bass_kernel_spmd(_CACHED["nc"], in_maps, list(range(8)))
    return assemble(res.results)


# revision 13
# speedup vs baseline: 1.7423x; 1.0361x over previous
"""Bilateral filter (7x7, dilation 1) Trainium2 Bass kernel — v7.

Problem: input [2, 18, 1024, 1024] f32.
  filterable = input[:, :8]; params = input[:, 8:]
  r_c = -(p_c^2), sx = -(p8^2), sy = -(p9^2)
  logw = sum_c r_c (fn_c - f_c)^2 + sx dx^2 + sy dy^2  (OOB taps masked)
  out[c] = sum_taps w * fn_c / sum_taps w,  c < 3

Sharding: data-parallel over (batch, H): 8 cores, each 256 rows of one batch
image (+3 halo rows each side, sentinel-padded host-side, sentinel=8).

Design (per core, 2 row-blocks x 2 W-chunks of [128 rows, 512 cols]):
  - fp16 on-chip compute; channel-planar free-axis layout [128, 8ch*518col]
    keeps every hot access-pattern unit-stride => DVE 2x_1P mode throughout.
  - Hot loop runs on DVE+ACT only.  GPSIMD tensor ops share the DVE SBUF
    port and throttle DVE ~1.8x while active (measured), so GPSIMD only
    does cast-DMA descriptor generation and memsets.
  - Per tap: DVE sub -> ACT Square (in-place) -> DVE m=p^2*q ->
    DVE pairwise-tree channel reduce -> +Asp -> ACT exp(scale=-1) ->
    DVE w*fn3 -> PE identity-matmul accumulates [w*fn3 | w] into PSUM
    (fp32) across all 49 taps.  4-stage software pipeline so every
    cross-engine dependency is issued >=1 full tap ahead.
  - Macro-boundary overlap: staging tiles are double-buffered and loaded
    (GPSIMD cast-DMA f32->f16) during the previous macro's taps; the 12
    partition-shifted SBUF->SBUF copies that build the 7 row-shifted tile
    sets are issued as soon as the tap loop stops reading each F tile.
  - Weights: w = exp(-(sum_c p_c^2 d_c^2 + a*sx^2 + b*sy^2)) with all
    terms positive; sentinel pixels drive the sum to ~1e4 so exp -> +0,
    reproducing the reference's OOB mask (no inf anywhere: max sum ~41K
    < fp16 max 65504).
"""

import sys

if "/opt/trn_rl_repo" not in sys.path:
    sys.path.insert(0, "/opt/trn_rl_repo")

import numpy as np

import concourse.bass as bass
import concourse.mybir as mybir
from concourse.bacc import Bacc
from concourse.tile import TileContext
from concourse.masks import make_identity

FP32 = mybir.dt.float32
FP16 = mybir.dt.float16
AF = mybir.ActivationFunctionType

B, C_ALL, H, W = 2, 18, 1024, 1024
CF = 8                      # filterable channels
CO = 3                      # output channels
KS, RAD = 7, 3
HC = H * B // 8             # 256 output rows per core
HIN = HC + 2 * RAD          # 262 input rows per core (halo padded host-side)
WC = 512                    # W chunk
NW = W // WC                # 2
NHB = HC // 128             # 2
WT = WC + 2 * RAD           # 518 (with column halo)
SENT = 8.0                  # sentinel: max quadratic form ~41K < fp16 max,
                            # so no inf on-chip, yet exp(-s) underflows to +0
D2IDX = [3, 2, 1, 0, 1, 2, 3]              # index into D2VALS: (k-3)^2
D2VALS = [0.0, 1.0, 4.0, 9.0]
CTR = KS // 2 * KS + KS // 2               # center tap (w == 1 fast path)

_CACHED = {}


def _cm(ap, w=WT, c=CF):
    """View flat [128, c*w] as [128, c, w] (channel-major blocks)."""
    return ap.rearrange("p (c x) -> p c x", c=c)


def build_nc():
    nc = Bacc()
    x = nc.dram_tensor("x", [HIN, C_ALL, W], FP32, kind="ExternalInput")
    y = nc.dram_tensor("y", [CO, HC, W], FP32, kind="ExternalOutput")

    macros = [(hb, wck) for hb in range(NHB) for wck in range(NW)]
    NM = len(macros)

    with TileContext(nc) as tc:
        with (
            tc.tile_pool(name="ipool", bufs=1) as ipool,
            tc.tile_pool(name="fpool", bufs=1) as fpool,
            tc.tile_pool(name="cpool", bufs=1) as cpool,
            tc.tile_pool(name="dpool", bufs=1) as dpool,
            tc.tile_pool(name="spool", bufs=1) as spool,
            tc.tile_pool(name="ppool", bufs=1, space="PSUM") as ppool,
        ):
            ident = ipool.tile([128, 128], FP16, tag="ident", name="ident")
            make_identity(nc, ident[:])

            st = {}     # macro idx -> (Ct, Fm, Cb) staging tiles
            Fk = {}     # macro idx -> {oy: tile}

            def col_range(wck):
                lo = RAD if wck == 0 else 0
                hi = WT - RAD if wck == NW - 1 else WT
                return lo, hi

            def load_staging(k):
                hb, wck = macros[k]
                w0, r0 = wck * WC, hb * 128
                lo, hi = col_range(wck)
                tiles = []
                for nm, rb, nr in (("Ct", r0, RAD), ("Fm", r0 + RAD, 128),
                                   ("Cb", r0 + RAD + 128, RAD)):
                    tile = fpool.tile([nr, CF * WT], FP16, tag=nm, bufs=2,
                                      name=f"{nm}_{k}")
                    v = tile[:].rearrange("p (c x) -> p c x", c=CF)
                    if lo > 0:
                        nc.gpsimd.memset(v[:, :, 0:lo], SENT)
                    if hi < WT:
                        nc.gpsimd.memset(v[:, :, hi:WT], SENT)
                    nc.gpsimd.dma_start(
                        out=v[:, :, lo:hi],
                        in_=x[rb : rb + nr, 0:CF, w0 - RAD + lo : w0 - RAD + hi],
                    )
                    tiles.append(tile)
                st[k] = tiles
                Fk[k] = {RAD: tiles[1]}

            def make_shift(k, oy):
                """Build row-shifted tile F[oy][p] = staging row oy+p."""
                Ct, Fm, Cb = st[k]
                Ft = fpool.tile([128, CF * WT], FP16, tag=f"F{oy}", bufs=1,
                                name=f"F{oy}_{k}")
                if oy < RAD:
                    kk = RAD - oy
                    nc.sync.dma_start(out=Ft[0:kk, :], in_=Ct[oy:RAD, :])
                    nc.sync.dma_start(out=Ft[kk:128, :], in_=Fm[0 : 128 - kk, :])
                else:
                    kk = oy - RAD
                    nc.sync.dma_start(out=Ft[0 : 128 - kk, :], in_=Fm[kk:128, :])
                    nc.sync.dma_start(out=Ft[128 - kk : 128, :],
                                      in_=Cb[0:kk, :])
                Fk[k][oy] = Ft

            load_staging(0)
            for oy in range(KS):
                if oy != RAD:
                    make_shift(0, oy)

            for k in range(NM):
                if k + 1 < NM:
                    load_staging(k + 1)   # cast-DMAs overlap this macro
                _macro(nc, tc, x, y, ident, fpool, cpool, dpool, spool,
                       ppool, macros, k, Fk, make_shift)
    nc.compile()
    return nc


def _macro(nc, tc, x, y, ident, fpool, cpool, dpool, spool, ppool,
           macros, k, Fk, make_shift):
    hb, wck = macros[k]
    NM = len(macros)
    w0 = wck * WC
    r0 = hb * 128
    F = Fk[k]
    Fc = _cm(F[RAD][:])[:, :, RAD : RAD + WC]

    # ---- params: P2[c] = p_c^2 (f16, c-major), sx2/sy2 ----
    # DMAs ride the ACT queue (HWDGE) so they never serialize behind the
    # F-tile shift copies on the sync queue.
    P2 = cpool.tile([128, CF * WC], FP16, tag="P2", name=f"P2_{k}")
    sxy2 = cpool.tile([128, 2 * WC], FP16, tag="sxy2", name=f"sxy2_{k}")
    for kk in range(CF + 2):
        pst = fpool.tile([128, WC], FP32, tag="pst", bufs=2,
                         name=f"pst_{k}_{kk}")
        nc.scalar.dma_start(
            out=pst[:],
            in_=x[r0 + RAD : r0 + RAD + 128, CF + kk, w0 : w0 + WC])
        dst = (P2[:, kk * WC : (kk + 1) * WC] if kk < CF
               else sxy2[:, (kk - CF) * WC : (kk - CF + 1) * WC])
        nc.scalar.activation(dst, pst[:], AF.Square)
    sx2 = sxy2[:, 0:WC]
    sy2 = sxy2[:, WC : 2 * WC]

    # ---- spatial log-weights: asp(a, b) = a*sx2 + b*sy2 (positive) ----
    Aa = cpool.tile([128, 3 * WC], FP16, tag="Aa", name=f"Aa_{k}")
    Ab = cpool.tile([128, 3 * WC], FP16, tag="Ab", name=f"Ab_{k}")
    for ai in (1, 2, 3):
        nc.vector.tensor_scalar_mul(
            Aa[:, (ai - 1) * WC : ai * WC], sx2, float(D2VALS[ai]))
        nc.vector.tensor_scalar_mul(
            Ab[:, (ai - 1) * WC : ai * WC], sy2, float(D2VALS[ai]))
    Asum = cpool.tile([128, 9 * WC], FP16, tag="Asum", name=f"As_{k}")
    for ai in (1, 2, 3):
        for bi in (1, 2, 3):
            kk = (ai - 1) * 3 + (bi - 1)
            nc.vector.tensor_add(
                Asum[:, kk * WC : (kk + 1) * WC],
                Aa[:, (ai - 1) * WC : ai * WC],
                Ab[:, (bi - 1) * WC : bi * WC])

    def asp_ap(i, j):
        ai, bi = D2IDX[j], D2IDX[i]   # x-dist from col shift j, y from row i
        if ai == 0 and bi == 0:
            return None
        if bi == 0:
            return Aa[:, (ai - 1) * WC : ai * WC]
        if ai == 0:
            return Ab[:, (bi - 1) * WC : bi * WC]
        kk = (ai - 1) * 3 + (bi - 1)
        return Asum[:, kk * WC : (kk + 1) * WC]

    # ---- PSUM accumulator: [w*fn0 | w*fn1 | w*fn2 | w] ----
    ps = ppool.tile([128, 4 * WC], FP32, tag="ps", bufs=2, name=f"ps_{k}")

    taps = [(i, j) for i in range(KS) for j in range(KS)]
    n = len(taps)
    Dt, Tt = {}, {}

    def stage_a(t):     # sub + square (in-place)
        if t == CTR:
            return
        i, j = taps[t]
        d = dpool.tile([128, CF * WC], FP16, tag="d", bufs=5,
                       name=f"d_{k}_{t}")
        nc.vector.tensor_sub(_cm(d[:], WC), _cm(F[i][:])[:, :, j : j + WC], Fc)
        nc.scalar.activation(d[:], d[:], AF.Square)
        Dt[t] = d

    def stage_m(t):     # m = p^2 * q  (in-place)
        if t == CTR:
            return
        dv = Dt[t][:]
        nc.vector.tensor_mul(dv[:], P2[:], dv[:])

    def stage_r(t):     # tree-reduce, +asp, exp
        T = spool.tile([128, 4 * WC], FP16, tag="T", bufs=5,
                       name=f"T_{k}_{t}")
        Tt[t] = T
        if t == CTR:
            nc.gpsimd.memset(T[:, 3 * WC : 4 * WC], 1.0)
            return
        i, j = taps[t]
        dv = Dt.pop(t)[:]
        nc.vector.tensor_add(dv[:, 0 : 4 * WC], dv[:, 0 : 4 * WC],
                             dv[:, 4 * WC : 8 * WC])
        nc.vector.tensor_add(dv[:, 0 : 2 * WC], dv[:, 0 : 2 * WC],
                             dv[:, 2 * WC : 4 * WC])
        nc.vector.tensor_add(dv[:, 0:WC], dv[:, 0:WC], dv[:, WC : 2 * WC])
        ap = asp_ap(i, j)
        if ap is not None:
            nc.vector.tensor_add(dv[:, 0:WC], dv[:, 0:WC], ap)
        nc.scalar.activation(T[:, 3 * WC : 4 * WC], dv[:, 0:WC], AF.Exp,
                             scale=-1.0)

    def stage_c(t):     # w*fn3, then PE accumulates [w*fn3 | w] into PSUM
        i, j = taps[t]
        T = Tt.pop(t)
        fn3 = _cm(F[i][:])[:, 0:CO, j : j + WC]
        if t == CTR:
            nc.vector.tensor_copy(_cm(T[:, 0 : CO * WC], WC, CO), fn3)
        else:
            w_b = T[:, 3 * WC : 4 * WC].unsqueeze(1).broadcast_to(
                [128, CO, WC])
            nc.vector.tensor_mul(_cm(T[:, 0 : CO * WC], WC, CO), w_b, fn3)
        for kk in range(4):
            nc.tensor.matmul(
                ps[:, kk * WC : (kk + 1) * WC], ident[:],
                T[:, kk * WC : (kk + 1) * WC],
                start=(t == 0), stop=(t == n - 1))

    # exp (in stage_r) must precede the next square (stage_a) in the ACT
    # queue, else everything downstream of exp serializes behind the square.
    for t in range(n + 3):
        if 0 <= t - 2 < n:
            stage_r(t - 2)
        if t < n:
            stage_a(t)
        if 0 <= t - 1 < n:
            stage_m(t - 1)
        if 0 <= t - 3 < n:
            stage_c(t - 3)
        # Row block i of F is last read by stage_c(i*7+6), issued at
        # iteration i*7+9: rebuild it for the next macro right after.
        if k + 1 < NM and t >= 9 and (t - 9) % KS == 0:
            i_freed = (t - 9) // KS
            if i_freed < KS and i_freed != RAD:
                make_shift(k + 1, i_freed)

    # ---- out = acc / wsum ----
    rec = spool.tile([128, WC], FP32, tag="rec", name=f"rec_{k}")
    nc.vector.reciprocal(rec[:], ps[:, 3 * WC : 4 * WC])
    out3 = spool.tile([128, CO * WC], FP32, tag="out3", name=f"o3_{k}")
    rec_b = rec[:].unsqueeze(1).broadcast_to([128, CO, WC])
    nc.vector.tensor_mul(_cm(out3[:], WC, CO), rec_b,
                         _cm(ps[:, 0 : CO * WC], WC, CO))
    for c in range(CO):
        nc.sync.dma_start(out=y[c, r0 : r0 + 128, w0 : w0 + WC],
                          in_=out3[:, c * WC : (c + 1) * WC])


def shard_inputs(input):
    """input [2,18,1024,1024] -> 8 per-core slabs [262, 18, 1024]."""
    input = np.asarray(input, dtype=np.float32)
    per_b = 4
    rows = H // per_b
    in_maps = []
    for core in range(8):
        b, q = divmod(core, per_b)
        r0 = q * rows
        slab = np.full((HIN, C_ALL, W), SENT, dtype=np.float32)
        s_lo = max(r0 - RAD, 0)
        s_hi = min(r0 + rows + RAD, H)
        slab[s_lo - (r0 - RAD) : s_hi - (r0 - RAD), :, :] = (
            input[b, :, s_lo:s_hi, :].transpose(1, 0, 2))
        in_maps.append({"x": np.ascontiguousarray(slab)})
    return in_maps


def assemble(results):
    out = np.empty((B, CO, H, W), dtype=np.float32)
    rows = H // 4
    for core in range(8):
        b, q = divmod(core, 4)
        out[b, :, q * rows : (q + 1) * rows, :] = results[core]["y"]
    return out


def kernel(input):
    from concourse.bass_utils import run_bass_kernel_spmd

    if "nc" not in _CACHED:
        _CACHED["nc"] = build_nc()
    in_maps = shard_inputs(input)
    res = run_bass_kernel_spmd(_CACHED["nc"], in_maps, list(range(8)))
    return assemble(res.results)


# revision 16
# speedup vs baseline: 1.7485x; 1.0036x over previous
"""Bilateral filter (7x7, dilation 1) Trainium2 Bass kernel — v7.

Problem: input [2, 18, 1024, 1024] f32.
  filterable = input[:, :8]; params = input[:, 8:]
  r_c = -(p_c^2), sx = -(p8^2), sy = -(p9^2)
  logw = sum_c r_c (fn_c - f_c)^2 + sx dx^2 + sy dy^2  (OOB taps masked)
  out[c] = sum_taps w * fn_c / sum_taps w,  c < 3

Sharding: data-parallel over (batch, H): 8 cores, each 256 rows of one batch
image (+3 halo rows each side, sentinel-padded host-side, sentinel=8).

Design (per core, 2 row-blocks x 2 W-chunks of [128 rows, 512 cols]):
  - fp16 on-chip compute; channel-planar free-axis layout [128, 8ch*518col]
    keeps every hot access-pattern unit-stride => DVE 2x_1P mode throughout.
  - Hot loop runs on DVE+ACT only.  GPSIMD tensor ops share the DVE SBUF
    port and throttle DVE ~1.8x while active (measured), so GPSIMD only
    does cast-DMA descriptor generation and memsets.
  - Per tap: DVE sub -> ACT Square (in-place) -> DVE m=p^2*q ->
    DVE pairwise-tree channel reduce -> +Asp -> ACT exp(scale=-1) ->
    DVE w*fn3 -> PE identity-matmul accumulates [w*fn3 | w] into PSUM
    (fp32) across all 49 taps.  4-stage software pipeline so every
    cross-engine dependency is issued >=1 full tap ahead.
  - Macro-boundary overlap: staging tiles are double-buffered and loaded
    (GPSIMD cast-DMA f32->f16) during the previous macro's taps; the 12
    partition-shifted SBUF->SBUF copies that build the 7 row-shifted tile
    sets are issued as soon as the tap loop stops reading each F tile.
  - Weights: w = exp(-(sum_c p_c^2 d_c^2 + a*sx^2 + b*sy^2)) with all
    terms positive; sentinel pixels drive the sum to ~1e4 so exp -> +0,
    reproducing the reference's OOB mask (no inf anywhere: max sum ~41K
    < fp16 max 65504).
"""

import sys

if "/opt/trn_rl_repo" not in sys.path:
    sys.path.insert(0, "/opt/trn_rl_repo")

import numpy as np

import concourse.bass as bass
import concourse.mybir as mybir
from concourse.bacc import Bacc
from concourse.tile import TileContext
from concourse.masks import make_identity

FP32 = mybir.dt.float32
FP16 = mybir.dt.float16
AF = mybir.ActivationFunctionType

B, C_ALL, H, W = 2, 18, 1024, 1024
CF = 8                      # filterable channels
CO = 3                      # output channels
KS, RAD = 7, 3
HC = H * B // 8             # 256 output rows per core
HIN = HC + 2 * RAD          # 262 input rows per core (halo padded host-side)
WC = 512                    # W chunk
NW = W // WC                # 2
NHB = HC // 128             # 2
WT = WC + 2 * RAD           # 518 (with column halo)
SENT = 8.0                  # sentinel: max quadratic form ~41K < fp16 max,
                            # so no inf on-chip, yet exp(-s) underflows to +0
D2IDX = [3, 2, 1, 0, 1, 2, 3]              # index into D2VALS: (k-3)^2
D2VALS = [0.0, 1.0, 4.0, 9.0]
CTR = KS // 2 * KS + KS // 2               # center tap (w == 1 fast path)

_CACHED = {}


def _cm(ap, w=WT, c=CF):
    """View flat [128, c*w] as [128, c, w] (channel-major blocks)."""
    return ap.rearrange("p (c x) -> p c x", c=c)


def build_nc():
    nc = Bacc()
    x = nc.dram_tensor("x", [HIN, C_ALL, W], FP32, kind="ExternalInput")
    y = nc.dram_tensor("y", [CO, HC, W], FP32, kind="ExternalOutput")

    macros = [(hb, wck) for hb in range(NHB) for wck in range(NW)]
    NM = len(macros)

    with TileContext(nc) as tc:
        with (
            tc.tile_pool(name="ipool", bufs=1) as ipool,
            tc.tile_pool(name="fpool", bufs=1) as fpool,
            tc.tile_pool(name="cpool", bufs=1) as cpool,
            tc.tile_pool(name="dpool", bufs=1) as dpool,
            tc.tile_pool(name="spool", bufs=1) as spool,
            tc.tile_pool(name="ppool", bufs=1, space="PSUM") as ppool,
        ):
            ident = ipool.tile([128, 128], FP16, tag="ident", name="ident")
            make_identity(nc, ident[:])

            Fk = {}     # macro idx -> {oy: tile}

            def make_ftile(k, oy):
                """F[oy][p] = slab row r0+oy+p, cast f32->f16 straight from
                DRAM (write-only SBUF traffic: no staging, no SBUF->SBUF
                copies contending with DVE reads)."""
                hb, wck = macros[k]
                w0, r0 = wck * WC, hb * 128
                lo = RAD if wck == 0 else 0
                hi = WT - RAD if wck == NW - 1 else WT
                Ft = fpool.tile([128, CF * WT], FP16, tag=f"F{oy}",
                                bufs=2 if oy == RAD else 1,
                                name=f"F{oy}_{k}")
                v = Ft[:].rearrange("p (c x) -> p c x", c=CF)
                if lo > 0:
                    nc.gpsimd.memset(v[:, :, 0:lo], SENT)
                if hi < WT:
                    nc.gpsimd.memset(v[:, :, hi:WT], SENT)
                nc.gpsimd.dma_start(
                    out=v[:, :, lo:hi],
                    in_=x[r0 + oy : r0 + oy + 128, 0:CF,
                          w0 - RAD + lo : w0 - RAD + hi],
                )
                Fk.setdefault(k, {})[oy] = Ft

            for oy in range(KS):
                make_ftile(0, oy)

            for k in range(NM):
                _macro(nc, tc, x, y, ident, fpool, cpool, dpool, spool,
                       ppool, macros, k, Fk, make_ftile)
    nc.compile()
    return nc


def _macro(nc, tc, x, y, ident, fpool, cpool, dpool, spool, ppool,
           macros, k, Fk, make_ftile):
    hb, wck = macros[k]
    NM = len(macros)
    w0 = wck * WC
    r0 = hb * 128
    F = Fk[k]
    Fc = _cm(F[RAD][:])[:, :, RAD : RAD + WC]

    # ---- params: P2[c] = p_c^2 (f16, c-major), sx2/sy2 ----
    # DMAs ride the ACT queue (HWDGE) so they never serialize behind the
    # F-tile shift copies on the sync queue.
    P2 = cpool.tile([128, CF * WC], FP16, tag="P2", name=f"P2_{k}")
    sxy2 = cpool.tile([128, 2 * WC], FP16, tag="sxy2", name=f"sxy2_{k}")
    for kk in range(CF + 2):
        pst = fpool.tile([128, WC], FP32, tag="pst", bufs=2,
                         name=f"pst_{k}_{kk}")
        nc.scalar.dma_start(
            out=pst[:],
            in_=x[r0 + RAD : r0 + RAD + 128, CF + kk, w0 : w0 + WC])
        dst = (P2[:, kk * WC : (kk + 1) * WC] if kk < CF
               else sxy2[:, (kk - CF) * WC : (kk - CF + 1) * WC])
        nc.scalar.activation(dst, pst[:], AF.Square)
    sx2 = sxy2[:, 0:WC]
    sy2 = sxy2[:, WC : 2 * WC]

    # ---- spatial log-weights: asp(a, b) = a*sx2 + b*sy2 (positive) ----
    Aa = cpool.tile([128, 3 * WC], FP16, tag="Aa", name=f"Aa_{k}")
    Ab = cpool.tile([128, 3 * WC], FP16, tag="Ab", name=f"Ab_{k}")
    for ai in (1, 2, 3):
        nc.vector.tensor_scalar_mul(
            Aa[:, (ai - 1) * WC : ai * WC], sx2, float(D2VALS[ai]))
        nc.vector.tensor_scalar_mul(
            Ab[:, (ai - 1) * WC : ai * WC], sy2, float(D2VALS[ai]))
    Asum = cpool.tile([128, 9 * WC], FP16, tag="Asum", name=f"As_{k}")
    for ai in (1, 2, 3):
        for bi in (1, 2, 3):
            kk = (ai - 1) * 3 + (bi - 1)
            nc.vector.tensor_add(
                Asum[:, kk * WC : (kk + 1) * WC],
                Aa[:, (ai - 1) * WC : ai * WC],
                Ab[:, (bi - 1) * WC : bi * WC])

    def asp_ap(i, j):
        ai, bi = D2IDX[j], D2IDX[i]   # x-dist from col shift j, y from row i
        if ai == 0 and bi == 0:
            return None
        if bi == 0:
            return Aa[:, (ai - 1) * WC : ai * WC]
        if ai == 0:
            return Ab[:, (bi - 1) * WC : bi * WC]
        kk = (ai - 1) * 3 + (bi - 1)
        return Asum[:, kk * WC : (kk + 1) * WC]

    # ---- PSUM accumulator: [w*fn0 | w*fn1 | w*fn2 | w] ----
    ps = ppool.tile([128, 4 * WC], FP32, tag="ps", bufs=2, name=f"ps_{k}")

    taps = [(i, j) for i in range(KS) for j in range(KS)]
    n = len(taps)
    Dt, Tt = {}, {}

    def stage_a(t):     # sub + square (in-place)
        if t == CTR:
            return
        i, j = taps[t]
        d = dpool.tile([128, CF * WC], FP16, tag="d", bufs=5,
                       name=f"d_{k}_{t}")
        nc.vector.tensor_sub(_cm(d[:], WC), _cm(F[i][:])[:, :, j : j + WC], Fc)
        nc.scalar.activation(d[:], d[:], AF.Square)
        Dt[t] = d

    def stage_m(t):     # m = p^2 * q  (in-place)
        if t == CTR:
            return
        dv = Dt[t][:]
        nc.vector.tensor_mul(dv[:], P2[:], dv[:])

    def stage_r(t):     # tree-reduce, +asp, exp
        T = spool.tile([128, 4 * WC], FP16, tag="T", bufs=5,
                       name=f"T_{k}_{t}")
        Tt[t] = T
        if t == CTR:
            nc.gpsimd.memset(T[:, 3 * WC : 4 * WC], 1.0)
            return
        i, j = taps[t]
        dv = Dt.pop(t)[:]
        nc.vector.tensor_add(dv[:, 0 : 4 * WC], dv[:, 0 : 4 * WC],
                             dv[:, 4 * WC : 8 * WC])
        nc.vector.tensor_add(dv[:, 0 : 2 * WC], dv[:, 0 : 2 * WC],
                             dv[:, 2 * WC : 4 * WC])
        nc.vector.tensor_add(dv[:, 0:WC], dv[:, 0:WC], dv[:, WC : 2 * WC])
        ap = asp_ap(i, j)
        if ap is not None:
            nc.vector.tensor_add(dv[:, 0:WC], dv[:, 0:WC], ap)
        nc.scalar.activation(T[:, 3 * WC : 4 * WC], dv[:, 0:WC], AF.Exp,
                             scale=-1.0)

    def stage_c(t):     # w*fn3, then PE accumulates [w*fn3 | w] into PSUM
        i, j = taps[t]
        T = Tt.pop(t)
        fn3 = _cm(F[i][:])[:, 0:CO, j : j + WC]
        if t == CTR:
            nc.vector.tensor_copy(_cm(T[:, 0 : CO * WC], WC, CO), fn3)
        else:
            w_b = T[:, 3 * WC : 4 * WC].unsqueeze(1).broadcast_to(
                [128, CO, WC])
            nc.vector.tensor_mul(_cm(T[:, 0 : CO * WC], WC, CO), w_b, fn3)
        for kk in range(4):
            nc.tensor.matmul(
                ps[:, kk * WC : (kk + 1) * WC], ident[:],
                T[:, kk * WC : (kk + 1) * WC],
                start=(t == 0), stop=(t == n - 1))

    # exp (in stage_r) must precede the next square (stage_a) in the ACT
    # queue, else everything downstream of exp serializes behind the square.
    for t in range(n + 3):
        if 0 <= t - 2 < n:
            stage_r(t - 2)
        if t < n:
            stage_a(t)
        if 0 <= t - 1 < n:
            stage_m(t - 1)
        if 0 <= t - 3 < n:
            stage_c(t - 3)
        # Row block i of F is last read by stage_c(i*7+6), issued at
        # iteration i*7+9: rebuild it for the next macro right after.
        # (F[3] is double-buffered, so its reload never blocks on Fc reads.)
        if k + 1 < NM and t >= 9 and (t - 9) % KS == 0:
            i_freed = (t - 9) // KS
            if i_freed < KS:
                make_ftile(k + 1, i_freed)

    # ---- out = acc / wsum ----
    rec = spool.tile([128, WC], FP32, tag="rec", name=f"rec_{k}")
    nc.vector.reciprocal(rec[:], ps[:, 3 * WC : 4 * WC])
    out3 = spool.tile([128, CO * WC], FP32, tag="out3", name=f"o3_{k}")
    rec_b = rec[:].unsqueeze(1).broadcast_to([128, CO, WC])
    nc.vector.tensor_mul(_cm(out3[:], WC, CO), rec_b,
                         _cm(ps[:, 0 : CO * WC], WC, CO))
    for c in range(CO):
        nc.sync.dma_start(out=y[c, r0 : r0 + 128, w0 : w0 + WC],
                          in_=out3[:, c * WC : (c + 1) * WC])


def shard_inputs(input):
    """input [2,18,1024,1024] -> 8 per-core slabs [262, 18, 1024]."""
    input = np.asarray(input, dtype=np.float32)
    per_b = 4
    rows = H // per_b
    in_maps = []
    for core in range(8):
        b, q = divmod(core, per_b)
        r0 = q * rows
        slab = np.full((HIN, C_ALL, W), SENT, dtype=np.float32)
        s_lo = max(r0 - RAD, 0)
        s_hi = min(r0 + rows + RAD, H)
        slab[s_lo - (r0 - RAD) : s_hi - (r0 - RAD), :, :] = (
            input[b, :, s_lo:s_hi, :].transpose(1, 0, 2))
        in_maps.append({"x": np.ascontiguousarray(slab)})
    return in_maps


def assemble(results):
    out = np.empty((B, CO, H, W), dtype=np.float32)
    rows = H // 4
    for core in range(8):
        b, q = divmod(core, 4)
        out[b, :, q * rows : (q + 1) * rows, :] = results[core]["y"]
    return out


def kernel(input):
    from concourse.bass_utils import run_bass_kernel_spmd

    if "nc" not in _CACHED:
        _CACHED["nc"] = build_nc()
    in_maps = shard_inputs(input)
    res = run_bass_kernel_spmd(_CACHED["nc"], in_maps, list(range(8)))
    return assemble(res.results)


# revision 17
# speedup vs baseline: 2.0898x; 1.1952x over previous
"""Bilateral filter (7x7, dilation 1) Trainium2 Bass kernel — v7.

Problem: input [2, 18, 1024, 1024] f32.
  filterable = input[:, :8]; params = input[:, 8:]
  r_c = -(p_c^2), sx = -(p8^2), sy = -(p9^2)
  logw = sum_c r_c (fn_c - f_c)^2 + sx dx^2 + sy dy^2  (OOB taps masked)
  out[c] = sum_taps w * fn_c / sum_taps w,  c < 3

Sharding: data-parallel over (batch, H): 8 cores, each 256 rows of one batch
image (+3 halo rows each side, sentinel-padded host-side, sentinel=8).

Design (per core, 2 row-blocks x 2 W-chunks of [128 rows, 512 cols]):
  - fp16 on-chip compute; channel-planar free-axis layout [128, 8ch*518col]
    keeps every hot access-pattern unit-stride => DVE 2x_1P mode throughout.
  - Hot loop runs on DVE+ACT only.  GPSIMD tensor ops share the DVE SBUF
    port and throttle DVE ~1.8x while active (measured), so GPSIMD only
    does cast-DMA descriptor generation and memsets.
  - Per tap: DVE sub -> ACT Square (in-place) -> DVE m=p^2*q ->
    DVE pairwise-tree channel reduce -> +Asp -> ACT exp(scale=-1) ->
    DVE w*fn3 -> PE identity-matmul accumulates [w*fn3 | w] into PSUM
    (fp32) across all 49 taps.  4-stage software pipeline so every
    cross-engine dependency is issued >=1 full tap ahead.
  - Macro-boundary overlap: staging tiles are double-buffered and loaded
    (GPSIMD cast-DMA f32->f16) during the previous macro's taps; the 12
    partition-shifted SBUF->SBUF copies that build the 7 row-shifted tile
    sets are issued as soon as the tap loop stops reading each F tile.
  - Weights: w = exp(-(sum_c p_c^2 d_c^2 + a*sx^2 + b*sy^2)) with all
    terms positive; sentinel pixels drive the sum to ~1e4 so exp -> +0,
    reproducing the reference's OOB mask (no inf anywhere: max sum ~41K
    < fp16 max 65504).
"""

import sys

if "/opt/trn_rl_repo" not in sys.path:
    sys.path.insert(0, "/opt/trn_rl_repo")

import numpy as np

import concourse.bass as bass
import concourse.mybir as mybir
from concourse.bacc import Bacc
from concourse.tile import TileContext
from concourse.masks import make_identity

FP32 = mybir.dt.float32
FP16 = mybir.dt.float16
AF = mybir.ActivationFunctionType

B, C_ALL, H, W = 2, 18, 1024, 1024
CF = 8                      # filterable channels
CO = 3                      # output channels
KS, RAD = 7, 3
HC = H * B // 8             # 256 output rows per core
HIN = HC + 2 * RAD          # 262 input rows per core (halo padded host-side)
WC = 512                    # W chunk
NW = W // WC                # 2
NHB = HC // 128             # 2
WT = WC + 2 * RAD           # 518 (with column halo)
SENT = 8.0                  # sentinel: max quadratic form ~41K < fp16 max,
                            # so no inf on-chip, yet exp(-s) underflows to +0
D2IDX = [3, 2, 1, 0, 1, 2, 3]              # index into D2VALS: (k-3)^2
D2VALS = [0.0, 1.0, 4.0, 9.0]
CTR = KS // 2 * KS + KS // 2               # center tap (w == 1 fast path)

_CACHED = {}


def _cm(ap, w=WT, c=CF):
    """View flat [128, c*w] as [128, c, w] (channel-major blocks)."""
    return ap.rearrange("p (c x) -> p c x", c=c)


def build_nc():
    nc = Bacc()
    x = nc.dram_tensor("x", [HIN, C_ALL, W], FP32, kind="ExternalInput")
    y = nc.dram_tensor("y", [CO, HC, W], FP32, kind="ExternalOutput")

    macros = [(hb, wck) for hb in range(NHB) for wck in range(NW)]
    NM = len(macros)

    with TileContext(nc) as tc:
        with (
            tc.tile_pool(name="ipool", bufs=1) as ipool,
            tc.tile_pool(name="fpool", bufs=1) as fpool,
            tc.tile_pool(name="cpool", bufs=1) as cpool,
            tc.tile_pool(name="dpool", bufs=1) as dpool,
            tc.tile_pool(name="spool", bufs=1) as spool,
            tc.tile_pool(name="ppool", bufs=1, space="PSUM") as ppool,
        ):
            ident = ipool.tile([128, 128], FP16, tag="ident", name="ident")
            make_identity(nc, ident[:])

            Fk = {}     # macro idx -> {oy: tile}

            def make_ftile(k, oy):
                """F[oy][p] = slab row r0+oy+p, cast f32->f16 straight from
                DRAM (write-only SBUF traffic: no staging, no SBUF->SBUF
                copies contending with DVE reads)."""
                hb, wck = macros[k]
                w0, r0 = wck * WC, hb * 128
                lo = RAD if wck == 0 else 0
                hi = WT - RAD if wck == NW - 1 else WT
                Ft = fpool.tile([128, CF * WT], FP16, tag=f"F{oy}",
                                bufs=2 if oy == RAD else 1,
                                name=f"F{oy}_{k}")
                v = Ft[:].rearrange("p (c x) -> p c x", c=CF)
                if lo > 0:
                    nc.gpsimd.memset(v[:, :, 0:lo], SENT)
                if hi < WT:
                    nc.gpsimd.memset(v[:, :, hi:WT], SENT)
                nc.gpsimd.dma_start(
                    out=v[:, :, lo:hi],
                    in_=x[r0 + oy : r0 + oy + 128, 0:CF,
                          w0 - RAD + lo : w0 - RAD + hi],
                )
                Fk.setdefault(k, {})[oy] = Ft

            for oy in range(KS):
                make_ftile(0, oy)

            for k in range(NM):
                _macro(nc, tc, x, y, ident, fpool, cpool, dpool, spool,
                       ppool, macros, k, Fk, make_ftile)
    nc.compile()
    return nc


def _macro(nc, tc, x, y, ident, fpool, cpool, dpool, spool, ppool,
           macros, k, Fk, make_ftile):
    hb, wck = macros[k]
    NM = len(macros)
    w0 = wck * WC
    r0 = hb * 128
    F = Fk[k]
    Fc = _cm(F[RAD][:])[:, :, RAD : RAD + WC]

    # ---- params: P2[c] = p_c^2 (f16, c-major), sx2/sy2 ----
    # DMAs ride the ACT queue (HWDGE) so they never serialize behind the
    # F-tile shift copies on the sync queue.
    P2 = cpool.tile([128, CF * WC], FP16, tag="P2", name=f"P2_{k}")
    sxy2 = cpool.tile([128, 2 * WC], FP16, tag="sxy2", name=f"sxy2_{k}")
    for kk in range(CF + 2):
        pst = fpool.tile([128, WC], FP32, tag="pst", bufs=2,
                         name=f"pst_{k}_{kk}")
        nc.scalar.dma_start(
            out=pst[:],
            in_=x[r0 + RAD : r0 + RAD + 128, CF + kk, w0 : w0 + WC])
        dst = (P2[:, kk * WC : (kk + 1) * WC] if kk < CF
               else sxy2[:, (kk - CF) * WC : (kk - CF + 1) * WC])
        nc.scalar.activation(dst, pst[:], AF.Square)
    sx2 = sxy2[:, 0:WC]
    sy2 = sxy2[:, WC : 2 * WC]

    # ---- spatial log-weights: asp(a, b) = a*sx2 + b*sy2 (positive) ----
    Aa = cpool.tile([128, 3 * WC], FP16, tag="Aa", name=f"Aa_{k}")
    Ab = cpool.tile([128, 3 * WC], FP16, tag="Ab", name=f"Ab_{k}")
    for ai in (1, 2, 3):
        nc.vector.tensor_scalar_mul(
            Aa[:, (ai - 1) * WC : ai * WC], sx2, float(D2VALS[ai]))
        nc.vector.tensor_scalar_mul(
            Ab[:, (ai - 1) * WC : ai * WC], sy2, float(D2VALS[ai]))
    Asum = cpool.tile([128, 9 * WC], FP16, tag="Asum", name=f"As_{k}")
    for ai in (1, 2, 3):
        for bi in (1, 2, 3):
            kk = (ai - 1) * 3 + (bi - 1)
            nc.vector.tensor_add(
                Asum[:, kk * WC : (kk + 1) * WC],
                Aa[:, (ai - 1) * WC : ai * WC],
                Ab[:, (bi - 1) * WC : bi * WC])

    def asp_ap(i, j):
        ai, bi = D2IDX[j], D2IDX[i]   # x-dist from col shift j, y from row i
        if ai == 0 and bi == 0:
            return None
        if bi == 0:
            return Aa[:, (ai - 1) * WC : ai * WC]
        if ai == 0:
            return Ab[:, (bi - 1) * WC : bi * WC]
        kk = (ai - 1) * 3 + (bi - 1)
        return Asum[:, kk * WC : (kk + 1) * WC]

    # ---- PSUM accumulator: [w*fn0 | w*fn1 | w*fn2 | w] ----
    ps = ppool.tile([128, 4 * WC], FP32, tag="ps", bufs=2, name=f"ps_{k}")

    taps = [(i, j) for i in range(KS) for j in range(KS)]
    n = len(taps)
    Dt, Tt = {}, {}

    def stage_sub(t):   # sub (independent of everything else in flight)
        if t == CTR:
            return
        i, j = taps[t]
        d = dpool.tile([128, CF * WC], FP16, tag="d", bufs=5,
                       name=f"d_{k}_{t}")
        nc.vector.tensor_sub(_cm(d[:], WC), _cm(F[i][:])[:, :, j : j + WC], Fc)
        Dt[t] = d

    def stage_sq(t):    # ACT square, in-place
        if t == CTR:
            return
        d = Dt[t]
        nc.scalar.activation(d[:], d[:], AF.Square)

    def stage_m(t):     # m = p^2 * q  (in-place)
        if t == CTR:
            return
        dv = Dt[t][:]
        nc.vector.tensor_mul(dv[:], P2[:], dv[:])

    def tree1(t):
        if t == CTR:
            return
        dv = Dt[t][:]
        nc.vector.tensor_add(dv[:, 0 : 4 * WC], dv[:, 0 : 4 * WC],
                             dv[:, 4 * WC : 8 * WC])

    def tree2(t):
        if t == CTR:
            return
        dv = Dt[t][:]
        nc.vector.tensor_add(dv[:, 0 : 2 * WC], dv[:, 0 : 2 * WC],
                             dv[:, 2 * WC : 4 * WC])

    def tree3(t):
        if t == CTR:
            return
        dv = Dt[t][:]
        nc.vector.tensor_add(dv[:, 0:WC], dv[:, 0:WC], dv[:, WC : 2 * WC])

    def stage_we(t):    # +asp, exp -> w
        T = spool.tile([128, 4 * WC], FP16, tag="T", bufs=5,
                       name=f"T_{k}_{t}")
        Tt[t] = T
        if t == CTR:
            nc.gpsimd.memset(T[:, 3 * WC : 4 * WC], 1.0)
            return
        i, j = taps[t]
        dv = Dt.pop(t)[:]
        ap = asp_ap(i, j)
        if ap is not None:
            nc.vector.tensor_add(dv[:, 0:WC], dv[:, 0:WC], ap)
        nc.scalar.activation(T[:, 3 * WC : 4 * WC], dv[:, 0:WC], AF.Exp,
                             scale=-1.0)

    def stage_c(t):     # w*fn3, then PE accumulates [w*fn3 | w] into PSUM
        i, j = taps[t]
        T = Tt.pop(t)
        fn3 = _cm(F[i][:])[:, 0:CO, j : j + WC]
        if t == CTR:
            nc.vector.tensor_copy(_cm(T[:, 0 : CO * WC], WC, CO), fn3)
        else:
            w_b = T[:, 3 * WC : 4 * WC].unsqueeze(1).broadcast_to(
                [128, CO, WC])
            nc.vector.tensor_mul(_cm(T[:, 0 : CO * WC], WC, CO), w_b, fn3)
        for kk in range(4):
            nc.tensor.matmul(
                ps[:, kk * WC : (kk + 1) * WC], ident[:],
                T[:, kk * WC : (kk + 1) * WC],
                start=(t == 0), stop=(t == n - 1))

    # DVE issue order interleaves the dependent tree chain of tap t-2 with
    # independent ops (sub of t, m of t-1, mul3 of t-3) so each op's pipe
    # DRAIN overlaps an unrelated op instead of stalling its consumer.
    # ACT order per iteration: exp(t-2) before square(t), so the exp->mul3
    # chain never queues behind a 3.7us square.
    for t in range(n + 3):
        if 0 <= t - 2 < n:
            tree1(t - 2)
        if t < n:
            stage_sub(t)
        if 0 <= t - 2 < n:
            tree2(t - 2)
        if 0 <= t - 1 < n:
            stage_m(t - 1)
        if 0 <= t - 2 < n:
            tree3(t - 2)
        if 0 <= t - 3 < n:
            stage_c(t - 3)
        if 0 <= t - 2 < n:
            stage_we(t - 2)
        if t < n:
            stage_sq(t)
        # Row block i of F is last read by stage_c(i*7+6), issued at
        # iteration i*7+9: rebuild it for the next macro right after.
        # (F[3] is double-buffered, so its reload never blocks on Fc reads.)
        if k + 1 < NM and t >= 9 and (t - 9) % KS == 0:
            i_freed = (t - 9) // KS
            if i_freed < KS:
                make_ftile(k + 1, i_freed)

    # ---- out = acc / wsum ----
    rec = spool.tile([128, WC], FP32, tag="rec", name=f"rec_{k}")
    nc.vector.reciprocal(rec[:], ps[:, 3 * WC : 4 * WC])
    out3 = spool.tile([128, CO * WC], FP32, tag="out3", name=f"o3_{k}")
    rec_b = rec[:].unsqueeze(1).broadcast_to([128, CO, WC])
    nc.vector.tensor_mul(_cm(out3[:], WC, CO), rec_b,
                         _cm(ps[:, 0 : CO * WC], WC, CO))
    for c in range(CO):
        nc.sync.dma_start(out=y[c, r0 : r0 + 128, w0 : w0 + WC],
                          in_=out3[:, c * WC : (c + 1) * WC])


def shard_inputs(input):
    """input [2,18,1024,1024] -> 8 per-core slabs [262, 18, 1024]."""
    input = np.asarray(input, dtype=np.float32)
    per_b = 4
    rows = H // per_b
    in_maps = []
    for core in range(8):
        b, q = divmod(core, per_b)
        r0 = q * rows
        slab = np.full((HIN, C_ALL, W), SENT, dtype=np.float32)
        s_lo = max(r0 - RAD, 0)
        s_hi = min(r0 + rows + RAD, H)
        slab[s_lo - (r0 - RAD) : s_hi - (r0 - RAD), :, :] = (
            input[b, :, s_lo:s_hi, :].transpose(1, 0, 2))
        in_maps.append({"x": np.ascontiguousarray(slab)})
    return in_maps


def assemble(results):
    out = np.empty((B, CO, H, W), dtype=np.float32)
    rows = H // 4
    for core in range(8):
        b, q = divmod(core, 4)
        out[b, :, q * rows : (q + 1) * rows, :] = results[core]["y"]
    return out


def kernel(input):
    from concourse.bass_utils import run_bass_kernel_spmd

    if "nc" not in _CACHED:
        _CACHED["nc"] = build_nc()
    in_maps = shard_inputs(input)
    res = run_bass_kernel_spmd(_CACHED["nc"], in_maps, list(range(8)))
    return assemble(res.results)


# revision 21
# speedup vs baseline: 2.1204x; 1.0146x over previous
"""Bilateral filter (7x7, dilation 1) Trainium2 Bass kernel — v7.

Problem: input [2, 18, 1024, 1024] f32.
  filterable = input[:, :8]; params = input[:, 8:]
  r_c = -(p_c^2), sx = -(p8^2), sy = -(p9^2)
  logw = sum_c r_c (fn_c - f_c)^2 + sx dx^2 + sy dy^2  (OOB taps masked)
  out[c] = sum_taps w * fn_c / sum_taps w,  c < 3

Sharding: data-parallel over (batch, H): 8 cores, each 256 rows of one batch
image (+3 halo rows each side, sentinel-padded host-side, sentinel=8).

Design (per core, 2 row-blocks x 2 W-chunks of [128 rows, 512 cols]):
  - fp16 on-chip compute; channel-planar free-axis layout [128, 8ch*518col]
    keeps every hot access-pattern unit-stride => DVE 2x_1P mode throughout.
  - Hot loop runs on DVE+ACT only.  GPSIMD tensor ops share the DVE SBUF
    port and throttle DVE ~1.8x while active (measured), so GPSIMD only
    does cast-DMA descriptor generation and memsets.
  - Per tap: DVE sub -> ACT Square (in-place) -> DVE m=p^2*q ->
    DVE pairwise-tree channel reduce -> +Asp -> ACT exp(scale=-1) ->
    DVE w*fn3 -> PE identity-matmul accumulates [w*fn3 | w] into PSUM
    (fp32) across all 49 taps.  4-stage software pipeline so every
    cross-engine dependency is issued >=1 full tap ahead.
  - Macro-boundary overlap: staging tiles are double-buffered and loaded
    (GPSIMD cast-DMA f32->f16) during the previous macro's taps; the 12
    partition-shifted SBUF->SBUF copies that build the 7 row-shifted tile
    sets are issued as soon as the tap loop stops reading each F tile.
  - Weights: w = exp(-(sum_c p_c^2 d_c^2 + a*sx^2 + b*sy^2)) with all
    terms positive; sentinel pixels drive the sum to ~1e4 so exp -> +0,
    reproducing the reference's OOB mask (no inf anywhere: max sum ~41K
    < fp16 max 65504).
"""

import sys

if "/opt/trn_rl_repo" not in sys.path:
    sys.path.insert(0, "/opt/trn_rl_repo")

import numpy as np

import concourse.bass as bass
import concourse.mybir as mybir
from concourse.bacc import Bacc
from concourse.tile import TileContext
from concourse.masks import make_identity

FP32 = mybir.dt.float32
FP16 = mybir.dt.float16
AF = mybir.ActivationFunctionType

B, C_ALL, H, W = 2, 18, 1024, 1024
CF = 8                      # filterable channels
CO = 3                      # output channels
KS, RAD = 7, 3
HC = H * B // 8             # 256 output rows per core
HIN = HC + 2 * RAD          # 262 input rows per core (halo padded host-side)
WC = 512                    # W chunk
NW = W // WC                # 2
NHB = HC // 128             # 2
WT = WC + 2 * RAD           # 518 (with column halo)
SENT = 8.0                  # sentinel: max quadratic form ~41K < fp16 max,
                            # so no inf on-chip, yet exp(-s) underflows to +0
D2IDX = [3, 2, 1, 0, 1, 2, 3]              # index into D2VALS: (k-3)^2
D2VALS = [0.0, 1.0, 4.0, 9.0]
CTR = KS // 2 * KS + KS // 2               # center tap (w == 1 fast path)

_CACHED = {}


def _cm(ap, w=WT, c=CF):
    """View flat [128, c*w] as [128, c, w] (channel-major blocks)."""
    return ap.rearrange("p (c x) -> p c x", c=c)


def build_nc():
    nc = Bacc()
    x = nc.dram_tensor("x", [HIN, C_ALL, W], FP32, kind="ExternalInput")
    y = nc.dram_tensor("y", [CO, HC, W], FP32, kind="ExternalOutput")

    macros = [(hb, wck) for hb in range(NHB) for wck in range(NW)]
    NM = len(macros)

    with TileContext(nc) as tc:
        with (
            tc.tile_pool(name="ipool", bufs=1) as ipool,
            tc.tile_pool(name="fpool", bufs=1) as fpool,
            tc.tile_pool(name="cpool", bufs=1) as cpool,
            tc.tile_pool(name="dpool", bufs=1) as dpool,
            tc.tile_pool(name="spool", bufs=1) as spool,
            tc.tile_pool(name="ppool", bufs=1, space="PSUM") as ppool,
        ):
            ident = ipool.tile([128, 128], FP16, tag="ident", name="ident")
            make_identity(nc, ident[:])

            Fk = {}     # macro idx -> {oy: tile}

            def make_ftile(k, oy):
                """F[oy][p] = slab row r0+oy+p, cast f32->f16 straight from
                DRAM (write-only SBUF traffic: no staging, no SBUF->SBUF
                copies contending with DVE reads)."""
                hb, wck = macros[k]
                w0, r0 = wck * WC, hb * 128
                lo = RAD if wck == 0 else 0
                hi = WT - RAD if wck == NW - 1 else WT
                Ft = fpool.tile([128, CF * WT], FP16, tag=f"F{oy}",
                                bufs=2 if oy == RAD else 1,
                                name=f"F{oy}_{k}")
                v = Ft[:].rearrange("p (c x) -> p c x", c=CF)
                if lo > 0:
                    nc.gpsimd.memset(v[:, :, 0:lo], SENT)
                if hi < WT:
                    nc.gpsimd.memset(v[:, :, hi:WT], SENT)
                nc.gpsimd.dma_start(
                    out=v[:, :, lo:hi],
                    in_=x[r0 + oy : r0 + oy + 128, 0:CF,
                          w0 - RAD + lo : w0 - RAD + hi],
                )
                Fk.setdefault(k, {})[oy] = Ft

            for oy in range(KS):
                make_ftile(0, oy)

            for k in range(NM):
                _macro(nc, tc, x, y, ident, fpool, cpool, dpool, spool,
                       ppool, macros, k, Fk, make_ftile)
    nc.compile()
    return nc


def _macro(nc, tc, x, y, ident, fpool, cpool, dpool, spool, ppool,
           macros, k, Fk, make_ftile):
    hb, wck = macros[k]
    NM = len(macros)
    w0 = wck * WC
    r0 = hb * 128
    F = Fk[k]
    Fc = _cm(F[RAD][:])[:, :, RAD : RAD + WC]

    # ---- params: P2[c] = p_c^2 (f16, c-major), sx2/sy2 ----
    # DMAs ride the ACT queue (HWDGE) so they never serialize behind the
    # F-tile shift copies on the sync queue.
    P2 = cpool.tile([128, CF * WC], FP16, tag="P2", name=f"P2_{k}")
    sxy2 = cpool.tile([128, 2 * WC], FP16, tag="sxy2", name=f"sxy2_{k}")
    for kk in range(CF + 2):
        pst = fpool.tile([128, WC], FP32, tag="pst", bufs=2,
                         name=f"pst_{k}_{kk}")
        nc.scalar.dma_start(
            out=pst[:],
            in_=x[r0 + RAD : r0 + RAD + 128, CF + kk, w0 : w0 + WC])
        dst = (P2[:, kk * WC : (kk + 1) * WC] if kk < CF
               else sxy2[:, (kk - CF) * WC : (kk - CF + 1) * WC])
        nc.scalar.activation(dst, pst[:], AF.Square)
    sx2 = sxy2[:, 0:WC]
    sy2 = sxy2[:, WC : 2 * WC]

    # ---- spatial log-weights, one tile of 16 slots: slot(bi*4+ai) =
    # D2VALS[ai]*sx2 + D2VALS[bi]*sy2.  Slot order is chosen so that the
    # two taps of every paired group land in adjacent (or equal) slots,
    # making the paired asp-add a plain contiguous slice. ----
    Asp16 = cpool.tile([128, 16 * WC], FP16, tag="Asp16", name=f"A16_{k}")
    for ai in (1, 2, 3):                       # bi = 0 row
        nc.vector.tensor_scalar_mul(
            Asp16[:, ai * WC : (ai + 1) * WC], sx2, float(D2VALS[ai]))
    for bi in (1, 2, 3):                       # ai = 0 column
        nc.vector.tensor_scalar_mul(
            Asp16[:, bi * 4 * WC : (bi * 4 + 1) * WC], sy2,
            float(D2VALS[bi]))
    for ai in (1, 2, 3):
        for bi in (1, 2, 3):
            s = bi * 4 + ai
            nc.vector.tensor_add(
                Asp16[:, s * WC : (s + 1) * WC],
                Asp16[:, bi * 4 * WC : (bi * 4 + 1) * WC],
                Asp16[:, ai * WC : (ai + 1) * WC])
    A3 = Asp16[:].rearrange("p (s x) -> p s x", s=16)

    def asp_slot(i, j):
        return D2IDX[i] * 4 + D2IDX[j]

    # ---- PSUM accumulator: [w*fn0 | w*fn1 | w*fn2 | w] ----
    ps = ppool.tile([128, 4 * WC], FP32, tag="ps", bufs=2, name=f"ps_{k}")

    # ---- tap groups: pairs of taps share one d-tile and run the square,
    # p^2-multiply, tree, asp and exp as single double-width ops, halving
    # the ~150ns fixed cost per DVE op.  Within a pair the taps are ordered
    # by asp slot so the paired asp-add reads adjacent slots. ----
    groups = []          # (i, [j...]) with len 1 or 2; None marks center
    row_last = {}
    for i in range(KS):
        gl = ([[0, 1], [2, 3], [4, 5], [6]] if i != RAD
              else [[0, 1], [2, 4], [5, 6], None])
        for js in gl:
            if js is None:
                groups.append((i, None))
            else:
                groups.append((i, sorted(js, key=lambda j: asp_slot(i, j))))
        row_last[i] = len(groups) - 1
    n = len(groups)
    DW = 2 * CF * WC     # d-tile width (two taps)
    TW = 2 * 4 * WC
    Dt, Tt = {}, {}

    def gv(ap, G, w):    # [128, G*w] -> [128, G, w]
        return ap.rearrange("p (g x) -> p g x", g=G)

    def stage_sub(g):    # per-tap subs into the halves of one shared tile
        i, js = groups[g]
        if js is None:
            return
        d = dpool.tile([128, DW], FP16, tag="d", bufs=3, name=f"d_{k}_{g}")
        for gi, j in enumerate(js):
            nc.vector.tensor_sub(
                _cm(d[:, gi * CF * WC : (gi + 1) * CF * WC], WC),
                _cm(F[i][:])[:, :, j : j + WC], Fc)
        Dt[g] = d

    def stage_sq(g):    # one ACT square over both halves, in-place
        i, js = groups[g]
        if js is None:
            return
        d = Dt[g]
        nc.scalar.activation(d[:, 0 : len(js) * CF * WC],
                             d[:, 0 : len(js) * CF * WC], AF.Square)

    def stage_m(g):     # m = p^2 * q over both halves (P2 broadcast)
        i, js = groups[g]
        if js is None:
            return
        G = len(js)
        dv = gv(Dt[g][:, 0 : G * CF * WC], G, CF * WC)
        p2b = P2[:].unsqueeze(1).broadcast_to([128, G, CF * WC])
        nc.vector.tensor_mul(dv, p2b, dv)

    def tree(g, lvl):   # halve each tap's channel block, both taps at once

        i, js = groups[g]
        if js is None:
            return
        G = len(js)
        hw = (CF >> (lvl - 1)) * WC       # block width entering this level
        dv = gv(Dt[g][:, 0 : G * CF * WC], G, CF * WC)
        nc.vector.tensor_add(dv[:, :, 0 : hw // 2], dv[:, :, 0 : hw // 2],
                             dv[:, :, hw // 2 : hw])

    def stage_we(g):    # +asp (paired slot read), exp -> w
        i, js = groups[g]
        T = spool.tile([128, TW], FP16, tag="T", bufs=3, name=f"T_{k}_{g}")
        Tt[g] = T
        if js is None:
            nc.gpsimd.memset(T[:, 3 * WC : 4 * WC], 1.0)
            return
        G = len(js)
        dv = gv(Dt.pop(g)[:, 0 : G * CF * WC], G, CF * WC)[:, :, 0:WC]
        s0 = asp_slot(i, js[0])
        if G == 2:
            s1 = asp_slot(i, js[1])
            aspv = (A3[:, s0 : s0 + 2, :] if s1 == s0 + 1
                    else A3[:, s0 : s0 + 1, :].broadcast_to([128, 2, WC]))
        else:
            aspv = A3[:, s0 : s0 + 1, :]
        nc.vector.tensor_add(dv, dv, aspv)
        tv = gv(T[:, 0 : G * 4 * WC], G, 4 * WC)
        nc.scalar.activation(tv[:, :, 3 * WC : 4 * WC], dv, AF.Exp,
                             scale=-1.0)

    def stage_c(g):     # w*fn3 per tap, PE accumulates [w*fn3 | w] chunks
        i, js = groups[g]
        T = Tt.pop(g)
        if js is None:
            fn3 = _cm(F[i][:])[:, 0:CO, RAD : RAD + WC]
            nc.vector.tensor_copy(_cm(T[:, 0 : CO * WC], WC, CO), fn3)
            js_eff = [RAD]
        else:
            js_eff = js
            for gi, j in enumerate(js):
                o = gi * 4 * WC
                w_b = T[:, o + 3 * WC : o + 4 * WC].unsqueeze(1).broadcast_to(
                    [128, CO, WC])
                fn3 = _cm(F[i][:])[:, 0:CO, j : j + WC]
                nc.vector.tensor_mul(
                    _cm(T[:, o : o + CO * WC], WC, CO), w_b, fn3)
        for gi in range(len(js_eff)):
            for kk in range(4):
                nc.tensor.matmul(
                    ps[:, kk * WC : (kk + 1) * WC], ident[:],
                    T[:, (gi * 4 + kk) * WC : (gi * 4 + kk + 1) * WC],
                    start=(g == 0 and gi == 0),
                    stop=(g == n - 1 and gi == len(js_eff) - 1))

    # DVE issue order interleaves the dependent tree chain of group g-2 with
    # independent ops (sub of g, m of g-1, mul3 of g-3) so each op's pipe
    # DRAIN overlaps an unrelated op instead of stalling its consumer.
    # ACT order per iteration: exp(g-2) before square(g), so the exp->mul3
    # chain never queues behind the big square.
    for g in range(n + 3):
        if 0 <= g - 2 < n:
            tree(g - 2, 1)
        if g < n:
            stage_sub(g)
        if 0 <= g - 2 < n:
            tree(g - 2, 2)
        if 0 <= g - 1 < n:
            stage_m(g - 1)
        if 0 <= g - 2 < n:
            tree(g - 2, 3)
        if 0 <= g - 3 < n:
            stage_c(g - 3)
        if 0 <= g - 2 < n:
            stage_we(g - 2)
        if g < n:
            stage_sq(g)
        # Row block i of F is last read by stage_c(row_last[i]), issued at
        # iteration row_last[i]+3: rebuild it for the next macro after that.
        if k + 1 < NM and 0 <= g - 3 < n:
            i_done, js_done = groups[g - 3]
            if g - 3 == row_last[i_done]:
                make_ftile(k + 1, i_done)

    # ---- out = acc / wsum ----
    rec = spool.tile([128, WC], FP32, tag="rec", name=f"rec_{k}")
    nc.vector.reciprocal(rec[:], ps[:, 3 * WC : 4 * WC])
    out3 = spool.tile([128, CO * WC], FP32, tag="out3", name=f"o3_{k}")
    rec_b = rec[:].unsqueeze(1).broadcast_to([128, CO, WC])
    nc.vector.tensor_mul(_cm(out3[:], WC, CO), rec_b,
                         _cm(ps[:, 0 : CO * WC], WC, CO))
    for c in range(CO):
        nc.sync.dma_start(out=y[c, r0 : r0 + 128, w0 : w0 + WC],
                          in_=out3[:, c * WC : (c + 1) * WC])


def shard_inputs(input):
    """input [2,18,1024,1024] -> 8 per-core slabs [262, 18, 1024]."""
    input = np.asarray(input, dtype=np.float32)
    per_b = 4
    rows = H // per_b
    in_maps = []
    for core in range(8):
        b, q = divmod(core, per_b)
        r0 = q * rows
        slab = np.full((HIN, C_ALL, W), SENT, dtype=np.float32)
        s_lo = max(r0 - RAD, 0)
        s_hi = min(r0 + rows + RAD, H)
        slab[s_lo - (r0 - RAD) : s_hi - (r0 - RAD), :, :] = (
            input[b, :, s_lo:s_hi, :].transpose(1, 0, 2))
        in_maps.append({"x": np.ascontiguousarray(slab)})
    return in_maps


def assemble(results):
    out = np.empty((B, CO, H, W), dtype=np.float32)
    rows = H // 4
    for core in range(8):
        b, q = divmod(core, 4)
        out[b, :, q * rows : (q + 1) * rows, :] = results[core]["y"]
    return out


def kernel(input):
    from concourse.bass_utils import run_bass_kernel_spmd

    if "nc" not in _CACHED:
        _CACHED["nc"] = build_nc()
    in_maps = shard_inputs(input)
    res = run_bass_kernel_spmd(_CACHED["nc"], in_maps, list(range(8)))
    return assemble(res.results)


# revision 22
# speedup vs baseline: 2.1338x; 1.0063x over previous
"""Bilateral filter (7x7, dilation 1) Trainium2 Bass kernel — v7.

Problem: input [2, 18, 1024, 1024] f32.
  filterable = input[:, :8]; params = input[:, 8:]
  r_c = -(p_c^2), sx = -(p8^2), sy = -(p9^2)
  logw = sum_c r_c (fn_c - f_c)^2 + sx dx^2 + sy dy^2  (OOB taps masked)
  out[c] = sum_taps w * fn_c / sum_taps w,  c < 3

Sharding: data-parallel over (batch, H): 8 cores, each 256 rows of one batch
image (+3 halo rows each side, sentinel-padded host-side, sentinel=8).

Design (per core, 2 row-blocks x 2 W-chunks of [128 rows, 512 cols]):
  - fp16 on-chip compute; channel-planar free-axis layout [128, 8ch*518col]
    keeps every hot access-pattern unit-stride => DVE 2x_1P mode throughout.
  - Hot loop runs on DVE+ACT only.  GPSIMD tensor ops share the DVE SBUF
    port and throttle DVE ~1.8x while active (measured), so GPSIMD only
    does cast-DMA descriptor generation and memsets.
  - Per tap: DVE sub -> ACT Square (in-place) -> DVE m=p^2*q ->
    DVE pairwise-tree channel reduce -> +Asp -> ACT exp(scale=-1) ->
    DVE w*fn3 -> PE identity-matmul accumulates [w*fn3 | w] into PSUM
    (fp32) across all 49 taps.  4-stage software pipeline so every
    cross-engine dependency is issued >=1 full tap ahead.
  - Macro-boundary overlap: staging tiles are double-buffered and loaded
    (GPSIMD cast-DMA f32->f16) during the previous macro's taps; the 12
    partition-shifted SBUF->SBUF copies that build the 7 row-shifted tile
    sets are issued as soon as the tap loop stops reading each F tile.
  - Weights: w = exp(-(sum_c p_c^2 d_c^2 + a*sx^2 + b*sy^2)) with all
    terms positive; sentinel pixels drive the sum to ~1e4 so exp -> +0,
    reproducing the reference's OOB mask (no inf anywhere: max sum ~41K
    < fp16 max 65504).
"""

import sys

if "/opt/trn_rl_repo" not in sys.path:
    sys.path.insert(0, "/opt/trn_rl_repo")

import numpy as np

import concourse.bass as bass
import concourse.mybir as mybir
from concourse.bacc import Bacc
from concourse.tile import TileContext
from concourse.masks import make_identity

FP32 = mybir.dt.float32
FP16 = mybir.dt.float16
AF = mybir.ActivationFunctionType

B, C_ALL, H, W = 2, 18, 1024, 1024
CF = 8                      # filterable channels
CO = 3                      # output channels
KS, RAD = 7, 3
HC = H * B // 8             # 256 output rows per core
HIN = HC + 2 * RAD          # 262 input rows per core (halo padded host-side)
WC = 512                    # W chunk
NW = W // WC                # 2
NHB = HC // 128             # 2
WT = WC + 2 * RAD           # 518 (with column halo)
SENT = 8.0                  # sentinel: max quadratic form ~41K < fp16 max,
                            # so no inf on-chip, yet exp(-s) underflows to +0
D2IDX = [3, 2, 1, 0, 1, 2, 3]              # index into D2VALS: (k-3)^2
D2VALS = [0.0, 1.0, 4.0, 9.0]
CTR = KS // 2 * KS + KS // 2               # center tap (w == 1 fast path)

_CACHED = {}


def _cm(ap, w=WT, c=CF):
    """View flat [128, c*w] as [128, c, w] (channel-major blocks)."""
    return ap.rearrange("p (c x) -> p c x", c=c)


def build_nc():
    nc = Bacc()
    x = nc.dram_tensor("x", [HIN, C_ALL, W], FP32, kind="ExternalInput")
    y = nc.dram_tensor("y", [CO, HC, W], FP32, kind="ExternalOutput")

    macros = [(hb, wck) for hb in range(NHB) for wck in range(NW)]
    NM = len(macros)

    with TileContext(nc) as tc:
        with (
            tc.tile_pool(name="ipool", bufs=1) as ipool,
            tc.tile_pool(name="fpool", bufs=1) as fpool,
            tc.tile_pool(name="cpool", bufs=1) as cpool,
            tc.tile_pool(name="dpool", bufs=1) as dpool,
            tc.tile_pool(name="spool", bufs=1) as spool,
            tc.tile_pool(name="ppool", bufs=1, space="PSUM") as ppool,
        ):
            ident = ipool.tile([128, 128], FP16, tag="ident", name="ident")
            make_identity(nc, ident[:])

            Fk = {}     # macro idx -> {oy: tile}

            def make_ftile(k, oy):
                """F[oy][p] = slab row r0+oy+p, cast f32->f16 straight from
                DRAM (write-only SBUF traffic: no staging, no SBUF->SBUF
                copies contending with DVE reads)."""
                hb, wck = macros[k]
                w0, r0 = wck * WC, hb * 128
                lo = RAD if wck == 0 else 0
                hi = WT - RAD if wck == NW - 1 else WT
                Ft = fpool.tile([128, CF * WT], FP16, tag=f"F{oy}",
                                bufs=2 if oy == RAD else 1,
                                name=f"F{oy}_{k}")
                v = Ft[:].rearrange("p (c x) -> p c x", c=CF)
                if lo > 0:
                    nc.gpsimd.memset(v[:, :, 0:lo], SENT)
                if hi < WT:
                    nc.gpsimd.memset(v[:, :, hi:WT], SENT)
                nc.gpsimd.dma_start(
                    out=v[:, :, lo:hi],
                    in_=x[r0 + oy : r0 + oy + 128, 0:CF,
                          w0 - RAD + lo : w0 - RAD + hi],
                )
                Fk.setdefault(k, {})[oy] = Ft

            for oy in range(KS):
                make_ftile(0, oy)

            for k in range(NM):
                _macro(nc, tc, x, y, ident, fpool, cpool, dpool, spool,
                       ppool, macros, k, Fk, make_ftile)
    nc.compile()
    return nc


def _macro(nc, tc, x, y, ident, fpool, cpool, dpool, spool, ppool,
           macros, k, Fk, make_ftile):
    hb, wck = macros[k]
    NM = len(macros)
    w0 = wck * WC
    r0 = hb * 128
    F = Fk[k]
    Fc = _cm(F[RAD][:])[:, :, RAD : RAD + WC]

    # ---- params: P2[c] = p_c^2 (f16, c-major), sx2/sy2 ----
    # DMAs ride the ACT queue (HWDGE) so they never serialize behind the
    # F-tile shift copies on the sync queue.
    P2 = cpool.tile([128, CF * WC], FP16, tag="P2", name=f"P2_{k}")
    sxy2 = cpool.tile([128, 2 * WC], FP16, tag="sxy2", name=f"sxy2_{k}")
    for kk in range(CF + 2):
        pst = fpool.tile([128, WC], FP32, tag="pst", bufs=2,
                         name=f"pst_{k}_{kk}")
        nc.scalar.dma_start(
            out=pst[:],
            in_=x[r0 + RAD : r0 + RAD + 128, CF + kk, w0 : w0 + WC])
        dst = (P2[:, kk * WC : (kk + 1) * WC] if kk < CF
               else sxy2[:, (kk - CF) * WC : (kk - CF + 1) * WC])
        nc.scalar.activation(dst, pst[:], AF.Square)
    sx2 = sxy2[:, 0:WC]
    sy2 = sxy2[:, WC : 2 * WC]

    # ---- spatial log-weights, one tile of 16 slots: slot(bi*4+ai) =
    # D2VALS[ai]*sx2 + D2VALS[bi]*sy2.  Slot order is chosen so that the
    # two taps of every paired group land in adjacent (or equal) slots,
    # making the paired asp-add a plain contiguous slice. ----
    Asp16 = cpool.tile([128, 16 * WC], FP16, tag="Asp16", name=f"A16_{k}")
    for ai in (1, 2, 3):                       # bi = 0 row
        nc.vector.tensor_scalar_mul(
            Asp16[:, ai * WC : (ai + 1) * WC], sx2, float(D2VALS[ai]))
    for bi in (1, 2, 3):                       # ai = 0 column
        nc.vector.tensor_scalar_mul(
            Asp16[:, bi * 4 * WC : (bi * 4 + 1) * WC], sy2,
            float(D2VALS[bi]))
    for ai in (1, 2, 3):
        for bi in (1, 2, 3):
            s = bi * 4 + ai
            nc.vector.tensor_add(
                Asp16[:, s * WC : (s + 1) * WC],
                Asp16[:, bi * 4 * WC : (bi * 4 + 1) * WC],
                Asp16[:, ai * WC : (ai + 1) * WC])
    A3 = Asp16[:].rearrange("p (s x) -> p s x", s=16)

    def asp_slot(i, j):
        return D2IDX[i] * 4 + D2IDX[j]

    # ---- PSUM accumulator: [w*fn0 | w*fn1 | w*fn2 | w] ----
    ps = ppool.tile([128, 4 * WC], FP32, tag="ps", bufs=2, name=f"ps_{k}")

    # ---- tap groups: pairs of taps share one d-tile and run the square,
    # p^2-multiply, tree, asp and exp as single double-width ops, halving
    # the ~150ns fixed cost per DVE op.  Within a pair the taps are ordered
    # by asp slot so the paired asp-add reads adjacent slots. ----
    groups = []          # (i, [j...]) with len 1 or 2; None marks center
    row_last = {}
    for i in range(KS):
        gl = ([[0, 1], [2, 3], [4, 5], [6]] if i != RAD
              else [[0, 1], [2, 4], [5, 6], None])
        for js in gl:
            if js is None:
                groups.append((i, None))
            else:
                groups.append((i, sorted(js, key=lambda j: asp_slot(i, j))))
        row_last[i] = len(groups) - 1
    n = len(groups)
    DW = 2 * CF * WC     # d-tile width (two taps)
    TW = 2 * 4 * WC
    Dt, Tt = {}, {}

    def gv(ap, G, w):    # [128, G*w] -> [128, G, w]
        return ap.rearrange("p (g x) -> p g x", g=G)

    def stage_sub(g):    # per-tap subs into the halves of one shared tile
        i, js = groups[g]
        if js is None:
            return
        d = dpool.tile([128, DW], FP16, tag="d", bufs=4, name=f"d_{k}_{g}")
        for gi, j in enumerate(js):
            nc.vector.tensor_sub(
                _cm(d[:, gi * CF * WC : (gi + 1) * CF * WC], WC),
                _cm(F[i][:])[:, :, j : j + WC], Fc)
        Dt[g] = d

    def stage_sq(g):    # one ACT square over both halves, in-place
        i, js = groups[g]
        if js is None:
            return
        d = Dt[g]
        nc.scalar.activation(d[:, 0 : len(js) * CF * WC],
                             d[:, 0 : len(js) * CF * WC], AF.Square)

    def stage_m(g):     # m = p^2 * q over both halves (P2 broadcast)
        i, js = groups[g]
        if js is None:
            return
        G = len(js)
        dv = gv(Dt[g][:, 0 : G * CF * WC], G, CF * WC)
        p2b = P2[:].unsqueeze(1).broadcast_to([128, G, CF * WC])
        nc.vector.tensor_mul(dv, p2b, dv)

    def tree(g, lvl):   # halve each tap's channel block, both taps at once

        i, js = groups[g]
        if js is None:
            return
        G = len(js)
        hw = (CF >> (lvl - 1)) * WC       # block width entering this level
        dv = gv(Dt[g][:, 0 : G * CF * WC], G, CF * WC)
        nc.vector.tensor_add(dv[:, :, 0 : hw // 2], dv[:, :, 0 : hw // 2],
                             dv[:, :, hw // 2 : hw])

    def stage_we(g):    # +asp (paired slot read), exp -> w
        i, js = groups[g]
        T = spool.tile([128, TW], FP16, tag="T", bufs=3, name=f"T_{k}_{g}")
        Tt[g] = T
        if js is None:
            nc.gpsimd.memset(T[:, 3 * WC : 4 * WC], 1.0)
            return
        G = len(js)
        dv = gv(Dt.pop(g)[:, 0 : G * CF * WC], G, CF * WC)[:, :, 0:WC]
        s0 = asp_slot(i, js[0])
        if G == 2:
            s1 = asp_slot(i, js[1])
            aspv = (A3[:, s0 : s0 + 2, :] if s1 == s0 + 1
                    else A3[:, s0 : s0 + 1, :].broadcast_to([128, 2, WC]))
        else:
            aspv = A3[:, s0 : s0 + 1, :]
        nc.vector.tensor_add(dv, dv, aspv)
        tv = gv(T[:, 0 : G * 4 * WC], G, 4 * WC)
        nc.scalar.activation(tv[:, :, 3 * WC : 4 * WC], dv, AF.Exp,
                             scale=-1.0)

    def stage_c(g):     # w*fn3 per tap, PE accumulates [w*fn3 | w] chunks
        i, js = groups[g]
        T = Tt.pop(g)
        if js is None:
            fn3 = _cm(F[i][:])[:, 0:CO, RAD : RAD + WC]
            nc.vector.tensor_copy(_cm(T[:, 0 : CO * WC], WC, CO), fn3)
            js_eff = [RAD]
        else:
            js_eff = js
            for gi, j in enumerate(js):
                o = gi * 4 * WC
                w_b = T[:, o + 3 * WC : o + 4 * WC].unsqueeze(1).broadcast_to(
                    [128, CO, WC])
                fn3 = _cm(F[i][:])[:, 0:CO, j : j + WC]
                nc.vector.tensor_mul(
                    _cm(T[:, o : o + CO * WC], WC, CO), w_b, fn3)
        for gi in range(len(js_eff)):
            for kk in range(4):
                nc.tensor.matmul(
                    ps[:, kk * WC : (kk + 1) * WC], ident[:],
                    T[:, (gi * 4 + kk) * WC : (gi * 4 + kk + 1) * WC],
                    start=(g == 0 and gi == 0),
                    stop=(g == n - 1 and gi == len(js_eff) - 1))

    # DVE issue order interleaves the dependent tree chain of group g-2 with
    # independent ops (sub of g, m of g-1, mul3 of g-3) so each op's pipe
    # DRAIN overlaps an unrelated op instead of stalling its consumer.
    # ACT order per iteration: exp(g-2) before square(g), so the exp->mul3
    # chain never queues behind the big square.
    for g in range(n + 3):
        if 0 <= g - 2 < n:
            tree(g - 2, 1)
        if g < n:
            stage_sub(g)
        if 0 <= g - 2 < n:
            tree(g - 2, 2)
        if 0 <= g - 1 < n:
            stage_m(g - 1)
        if 0 <= g - 2 < n:
            tree(g - 2, 3)
        if 0 <= g - 3 < n:
            stage_c(g - 3)
        if 0 <= g - 2 < n:
            stage_we(g - 2)
        if g < n:
            stage_sq(g)
        # Row block i of F is last read by stage_c(row_last[i]), issued at
        # iteration row_last[i]+3: rebuild it for the next macro after that.
        if k + 1 < NM and 0 <= g - 3 < n:
            i_done, js_done = groups[g - 3]
            if g - 3 == row_last[i_done]:
                make_ftile(k + 1, i_done)

    # ---- out = acc / wsum ----
    rec = spool.tile([128, WC], FP32, tag="rec", name=f"rec_{k}")
    nc.vector.reciprocal(rec[:], ps[:, 3 * WC : 4 * WC])
    out3 = spool.tile([128, CO * WC], FP32, tag="out3", name=f"o3_{k}")
    rec_b = rec[:].unsqueeze(1).broadcast_to([128, CO, WC])
    nc.vector.tensor_mul(_cm(out3[:], WC, CO), rec_b,
                         _cm(ps[:, 0 : CO * WC], WC, CO))
    for c in range(CO):
        nc.sync.dma_start(out=y[c, r0 : r0 + 128, w0 : w0 + WC],
                          in_=out3[:, c * WC : (c + 1) * WC])


def shard_inputs(input):
    """input [2,18,1024,1024] -> 8 per-core slabs [262, 18, 1024]."""
    input = np.asarray(input, dtype=np.float32)
    per_b = 4
    rows = H // per_b
    in_maps = []
    for core in range(8):
        b, q = divmod(core, per_b)
        r0 = q * rows
        slab = np.full((HIN, C_ALL, W), SENT, dtype=np.float32)
        s_lo = max(r0 - RAD, 0)
        s_hi = min(r0 + rows + RAD, H)
        slab[s_lo - (r0 - RAD) : s_hi - (r0 - RAD), :, :] = (
            input[b, :, s_lo:s_hi, :].transpose(1, 0, 2))
        in_maps.append({"x": np.ascontiguousarray(slab)})
    return in_maps


def assemble(results):
    out = np.empty((B, CO, H, W), dtype=np.float32)
    rows = H // 4
    for core in range(8):
        b, q = divmod(core, 4)
        out[b, :, q * rows : (q + 1) * rows, :] = results[core]["y"]
    return out


def kernel(input):
    from concourse.bass_utils import run_bass_kernel_spmd

    if "nc" not in _CACHED:
        _CACHED["nc"] = build_nc()
    in_maps = shard_inputs(input)
    res = run_bass_kernel_spmd(_CACHED["nc"], in_maps, list(range(8)))
    return assemble(res.results)
